# revision 8
# baseline (speedup 1.0000x reference)
"""LightGCN (LGN-DICE) forward loss on 8 Trainium2 NeuronCores.

Strategy (dst-sharded SpMM):
  - Pad node table to 150528 rows = 1176 blocks of 128. Core c owns dst blocks
    [147c, 147(c+1)) (rows [18816c, 18816(c+1))).
  - Fused table T0 = [emb_int | emb_pop] as [150528, 128] f32 (512B rows).
  - Each core takes the edges whose dst lies in its slice.  Edges are bucketed
    by (dst block, src range) where the 6 src ranges are overlapping 32768-row
    windows of the table (dma_gather indices are int16).  Each (block, range)
    run is padded to 5 chunks of 128 edges; each block has exactly 30 chunks
    -> fully uniform SPMD structure (one NEFF for all 8 cores; all raggedness
    lives in input data).
  - Per layer: dma_gather 512B rows of the norm-scaled table; DVE builds
    one-hots [128 edges x 128 dsts] from dst_local vs an iota row; PE matmuls
    accumulate the per-block [128, 128] aggregate in PSUM (30 matmuls/block);
    ACT evicts with per-partition norm scaling.
  - Between layers: AllGather of the norm^2-scaled layer-1 output slices
    builds the full layer-2 gather table.
  - Loss stage: feats3 = t0 + h1 + h2 (= 3*feats) per slice; discrepancy
    row-sums against host-built 0/1 indicator slices; batch rows gathered
    from the owning core's slice, scattered into a [12293, 128] buffer and
    AllReduce-summed; every core then computes the final 4 losses.
"""

import os

import numpy as np

# ---------------------------------------------------------------- constants
N_USER = 100000
N_ITEM = 50000
N = N_USER + N_ITEM          # 150000
D = 64
B = 4096
INT_W = 0.1
POP_W = 0.1
DIS_PEN = 0.01

CORES = 8
BLK = 128
BLK_PER_CORE = 147
SLICE = BLK * BLK_PER_CORE   # 18816
NPAD = SLICE * CORES         # 150528
NBLK = NPAD // BLK           # 1176

NRANGE = 6
RSTRIDE = 23552
RWIN = 32768                 # int16 index window
RBASE = [r * RSTRIDE for r in range(NRANGE)]     # last: 117760 (+32768 = 150528)

KBR = 5                      # chunks per (block, range) run
RUN = KBR * BLK              # 640 slots
CH_PER_BLK = NRANGE * KBR    # 30
CH_TOT = BLK_PER_CORE * CH_PER_BLK               # 4410 chunks per core
STREAM_L = BLK_PER_CORE * RUN                    # 94080 idx per range stream
CALL_CH = 15                 # chunks per dma_gather call
CALL_IDX = CALL_CH * BLK     # 1920
NCALL = STREAM_L // CALL_IDX                     # 49
assert NCALL * CALL_IDX == STREAM_L

PAD_DLOC = 200.0             # one-hot sentinel (no column matches)

# loss-stage row buffer: [u | ip | inn] role blocks of 4097 rows each,
# then trash row (pad scatter target) and stats row (discrepancy partials)
ROLE_OFF = [0, 4097, 2 * 4097]
ROW_TRASH = 3 * 4097         # 12291
ROW_STATS = ROW_TRASH + 1    # 12292
ROWS_N = ROW_STATS + 1       # 12293
BJ = B // BLK                # 32 batch column blocks


# =================================================================== host prep
def _wrap16(vals_i16):
    """Lay out a flat int16 idx list for dma_gather: [128, len/16] with idx i at
    (i % 16, i // 16), replicated across the 8 q7 core groups."""
    n = vals_i16.shape[0]
    assert n % 16 == 0
    w = vals_i16.reshape(n // 16, 16).T            # [16, n/16]
    return np.tile(w, (8, 1))                       # [128, n/16]


def _prep_core(c, src, dst, t0_full):
    """Build all per-core metadata arrays."""
    lo, hi = SLICE * c, SLICE * (c + 1)
    sel = (dst >= lo) & (dst < hi)
    s = src[sel].astype(np.int64)
    dl = (dst[sel] - lo).astype(np.int64)          # local dst 0..18815
    blk = dl >> 7
    # --- range assignment with rebalancing to <= RUN per (block, range).
    # Initial buckets are equal-width (25000 rows) so loads start uniform;
    # every bucket fits its 32768-row gather window: 25000(r+1)-23552r <= 32768.
    rng = np.minimum(s // (N // NRANGE), NRANGE - 1).astype(np.int64)
    for _sweep in range(6):
        cnt = np.bincount(blk * NRANGE + rng,
                          minlength=BLK_PER_CORE * NRANGE).reshape(
                              BLK_PER_CORE, NRANGE)
        over = np.argwhere(cnt > RUN)
        if len(over) == 0:
            break
        for b, r in over:
            excess = cnt[b, r] - RUN
            in_run = np.where((blk == b) & (rng == r))[0]
            for tgt in ([r - 1, r + 1] if r > 0 else [r + 1]):
                if excess <= 0 or not (0 <= tgt < NRANGE):
                    continue
                off = s[in_run] - RBASE[tgt]
                elig = in_run[(off >= 0) & (off < RWIN)]
                room = RUN - cnt[b, tgt]
                m = min(excess, len(elig), max(room, 0))
                if m > 0:
                    rng[elig[:m]] = tgt
                    cnt[b, tgt] += m
                    cnt[b, r] -= m
                    excess -= m
    cnt = np.bincount(blk * NRANGE + rng,
                      minlength=BLK_PER_CORE * NRANGE).reshape(
                          BLK_PER_CORE, NRANGE)
    assert cnt.max() <= RUN, f"core {c}: run overflow {cnt.max()} > {RUN}"

    # --- slot layout: stream r, block b occupies positions [RUN*b, RUN*(b+1))
    order = np.lexsort((dl, rng, blk))
    s, dl, rng = s[order], dl[order], rng[order]
    idx_streams = np.zeros((NRANGE, STREAM_L), np.int16)
    dstloc = np.full((BLK, CH_TOT), PAD_DLOC, np.float32)
    edge_pos = 0
    for b in range(BLK_PER_CORE):
        for r in range(NRANGE):
            nbr = cnt[b, r]
            if nbr:
                e_s = s[edge_pos:edge_pos + nbr]
                e_d = dl[edge_pos:edge_pos + nbr]
                base = RUN * b
                idx_streams[r, base:base + nbr] = (e_s - RBASE[r]).astype(np.int16)
                dloc128 = (e_d & 127).astype(np.float32)
                for k in range((nbr + BLK - 1) // BLK):
                    ci = b * CH_PER_BLK + r * KBR + k
                    seg = dloc128[k * BLK:(k + 1) * BLK]
                    dstloc[:len(seg), ci] = seg
                edge_pos += nbr
    assert edge_pos == len(s)

    idx16 = np.zeros((NRANGE, NCALL, BLK, CALL_IDX // 16), np.int16)
    for r in range(NRANGE):
        for q in range(NCALL):
            idx16[r, q] = _wrap16(idx_streams[r, q * CALL_IDX:(q + 1) * CALL_IDX])

    deg = np.bincount(dl, minlength=SLICE).astype(np.int32)
    deg_t = np.ascontiguousarray(deg.reshape(BLK_PER_CORE, BLK).T)  # [128, 147]

    return dict(idx16=idx16, dstloc=dstloc, deg=deg_t,
                t0_slice=np.ascontiguousarray(t0_full[lo:hi]))


def _prep_batch(user, item_p, item_n):
    """Batch row extraction metadata (per core) + indicator slices."""
    u = user.reshape(-1).astype(np.int64)
    ip = item_p.reshape(-1).astype(np.int64) + N_USER
    inn = item_n.reshape(-1).astype(np.int64) + N_USER
    g_all = np.concatenate([u, ip, inn])
    dest_all = np.concatenate([ROLE_OFF[k] + np.arange(B) for k in range(3)])
    owner = g_all // SLICE
    per_core = []
    for c in range(CORES):
        m = owner == c
        per_core.append((g_all[m] - SLICE * c, dest_all[m]))
    gmax = max(len(a) for a, _ in per_core)
    G_B = ((gmax + BLK - 1) // BLK) * BLK
    bidx16 = np.zeros((CORES, BLK, G_B // 16), np.int16)
    bpos = np.full((CORES, BLK, G_B // BLK), ROW_TRASH, np.int32)
    for c in range(CORES):
        loc, dest = per_core[c]
        v = np.zeros(G_B, np.int16)
        v[:len(loc)] = loc.astype(np.int16)
        bidx16[c] = _wrap16(v)
        p = np.full(G_B, ROW_TRASH, np.int32)
        p[:len(dest)] = dest
        bpos[c] = p.reshape(G_B // BLK, BLK).T
    item_nodes = np.unique(np.concatenate([ip, inn]))
    user_nodes = np.unique(u)
    ind = np.zeros((CORES, 2, BLK, BLK_PER_CORE), np.float32)
    for kind, nodes in enumerate((item_nodes, user_nodes)):
        cc = nodes // SLICE
        loc = nodes - cc * SLICE
        ind[cc, kind, loc & 127, loc >> 7] = 1.0
    return G_B, bidx16, bpos, ind, len(item_nodes), len(user_nodes)


# ============================================================== device program
def _build(nc, G_B, n_item_u, n_user_u):
    import concourse.bass as bass
    import concourse.mybir as mybir
    import concourse.tile as tile

    f32 = mybir.dt.float32
    i32 = mybir.dt.int32
    i16 = mybir.dt.int16
    FN = mybir.ActivationFunctionType
    OP = mybir.AluOpType
    X = mybir.AxisListType.X
    RG = [list(range(CORES))]

    # ---------------- I/O tensors
    t0 = nc.dram_tensor("t0", [NPAD, 2 * D], f32, kind="ExternalInput")
    t0s = nc.dram_tensor("t0s", [SLICE, 2 * D], f32, kind="ExternalInput")
    idx_in = nc.dram_tensor("idx16", [NRANGE, NCALL, BLK, CALL_IDX // 16], i16,
                            kind="ExternalInput")
    dstloc_in = nc.dram_tensor("dstloc", [BLK, CH_TOT], f32, kind="ExternalInput")
    deg_in = nc.dram_tensor("deg", [BLK, BLK_PER_CORE], i32, kind="ExternalInput")
    ind_in = nc.dram_tensor("ind", [2, BLK, BLK_PER_CORE], f32,
                            kind="ExternalInput")
    iota8_in = nc.dram_tensor("iota8", [BLK, 8 * BLK], f32, kind="ExternalInput")
    bidx_in = nc.dram_tensor("bidx16", [BLK, G_B // 16], i16, kind="ExternalInput")
    bpos_in = nc.dram_tensor("bpos", [BLK, G_B // BLK], i32, kind="ExternalInput")
    maskf_in = nc.dram_tensor("maskf", [BLK, BJ], f32, kind="ExternalInput")
    cvec_in = nc.dram_tensor("cvec", [4, 1], f32, kind="ExternalInput")
    dcoef_in = nc.dram_tensor("dcoef", [1, 2], f32, kind="ExternalInput")
    out4 = nc.dram_tensor("out4", [4], f32, kind="ExternalOutput")

    # ---------------- internal DRAM
    t0p = nc.dram_tensor("t0p", [NPAD, 2 * D], f32)        # scaled layer-1 table
    t1p = nc.dram_tensor("t1p", [NPAD, 2 * D], f32, addr_space="Shared")
    h1p_slice = nc.dram_tensor("h1p_slice", [SLICE, 2 * D], f32)
    h1_dram = nc.dram_tensor("h1_dram", [SLICE, 2 * D], f32)
    h2_dram = nc.dram_tensor("h2_dram", [SLICE, 2 * D], f32)
    feats_dram = nc.dram_tensor("feats_dram", [SLICE, 2 * D], f32)
    norm_sl_dram = nc.dram_tensor("norm_sl_dram", [SLICE], f32)
    norm_full_dram = nc.dram_tensor("norm_full_dram", [NPAD], f32,
                                    addr_space="Shared")
    rows_dram = nc.dram_tensor("rows_dram", [ROWS_N, 2 * D], f32)
    rows_full = nc.dram_tensor("rows_full", [ROWS_N, 2 * D], f32,
                               addr_space="Shared")
    scr_dram = nc.dram_tensor("scr_dram", [1, 1], f32)

    with tile.TileContext(nc) as tc:
        with tc.tile_pool(name="res", bufs=1) as res:
            # ======== phase 0: deg -> norm, AllGather norm
            ones_col = res.tile([BLK, 1], f32)
            nc.vector.memset(ones_col[:], 1.0)
            deg_i = res.tile([BLK, BLK_PER_CORE], i32)
            nc.sync.dma_start(out=deg_i[:], in_=deg_in[:, :])
            degf = res.tile([BLK, BLK_PER_CORE], f32)
            nc.vector.tensor_copy(out=degf[:], in_=deg_i[:])
            nc.vector.tensor_scalar_max(out=degf[:], in0=degf[:], scalar1=1.0)
            inv = res.tile([BLK, BLK_PER_CORE], f32)    # norm^2 = 1/max(deg,1)
            nc.vector.reciprocal(out=inv[:], in_=degf[:])
            norm = res.tile([BLK, BLK_PER_CORE], f32)
            nc.scalar.sqrt(out=norm[:], in_=inv[:])
            nc.sync.dma_start(
                out=norm_sl_dram.ap().rearrange("(b p) -> p b", p=BLK),
                in_=norm[:])
            nc.gpsimd.collective_compute(
                "AllGather", OP.bypass, replica_groups=RG,
                ins=[norm_sl_dram.ap()], outs=[norm_full_dram.ap()])
            norm_full = res.tile([BLK, NBLK], f32)
            nc.sync.dma_start(
                out=norm_full[:],
                in_=norm_full_dram.ap().rearrange("(t p) -> p t", p=BLK))

            # ======== phase 1: t0p = norm * t0 (full local table)
            GRP = 8
            t0_v = t0.ap().rearrange("(g t p) f -> g p t f", t=GRP, p=BLK)
            t0p_v = t0p.ap().rearrange("(g t p) f -> g p t f", t=GRP, p=BLK)
            with tc.tile_pool(name="sc", bufs=3) as sc:
                for g in range(NBLK // GRP):
                    tin = sc.tile([BLK, GRP, 2 * D], f32, tag="scin")
                    tout = sc.tile([BLK, GRP, 2 * D], f32, tag="scout")
                    nc.sync.dma_start(out=tin[:], in_=t0_v[g])
                    for t in range(GRP):
                        nc.scalar.mul(
                            out=tout[:, t, :], in_=tin[:, t, :],
                            mul=norm_full[:, g * GRP + t:g * GRP + t + 1])
                    nc.sync.dma_start(out=t0p_v[g], in_=tout[:])

            # ======== phase 2: the two propagation layers
            def layer(src_dram, lnum):
                with (
                    tc.tile_pool(name=f"l{lnum}cst", bufs=1) as cst,
                    tc.tile_pool(name=f"l{lnum}g0", bufs=2) as g0,
                    tc.tile_pool(name=f"l{lnum}g1", bufs=2) as g1,
                    tc.tile_pool(name=f"l{lnum}g2", bufs=2) as g2,
                    tc.tile_pool(name=f"l{lnum}g3", bufs=2) as g3,
                    tc.tile_pool(name=f"l{lnum}g4", bufs=2) as g4,
                    tc.tile_pool(name=f"l{lnum}g5", bufs=2) as g5,
                    tc.tile_pool(name=f"l{lnum}gi", bufs=4) as gidx_pool,
                    tc.tile_pool(name=f"l{lnum}oh", bufs=3) as oh_pool,
                    tc.tile_pool(name=f"l{lnum}ps", bufs=4, space="PSUM") as pp,
                    tc.tile_pool(name=f"l{lnum}ev", bufs=4) as ev_pool,
                ):
                    gpools = [g0, g1, g2, g3, g4, g5]
                    iota8 = cst.tile([BLK, 8, BLK], f32)
                    nc.sync.dma_start(
                        out=iota8[:],
                        in_=iota8_in.ap().rearrange("p (a q) -> p a q", a=8))
                    dstloc = cst.tile([BLK, CH_TOT], f32)
                    nc.sync.dma_start(out=dstloc[:], in_=dstloc_in[:, :])

                    gtiles = [[None] * NCALL for _ in range(NRANGE)]

                    def ensure_call(r, q):
                        if gtiles[r][q] is not None:
                            return
                        it = gidx_pool.tile([BLK, CALL_IDX // 16], i16,
                                            tag="gidx")
                        nc.sync.dma_start(out=it[:], in_=idx_in[r, q])
                        gt = gpools[r].tile([BLK, CALL_CH, BLK], f32,
                                            tag=f"g{r}")
                        nc.gpsimd.dma_gather(
                            out_ap=gt[:],
                            in_ap=src_dram.ap()[RBASE[r]:RBASE[r] + RWIN, :],
                            idxs_ap=it[:],
                            num_idxs=CALL_IDX,
                            num_idxs_reg=CALL_IDX,
                            elem_size=2 * D,
                            single_packet=False,
                        )
                        gtiles[r][q] = gt

                    oh8 = None
                    for b in range(BLK_PER_CORE):
                        pt = pp.tile([BLK, BLK], f32, tag="acc")
                        for r in range(NRANGE):
                            for k in range(KBR):
                                ci = b * CH_PER_BLK + r * KBR + k
                                if ci % 8 == 0:
                                    hi = min(ci + 8, CH_TOT)
                                    oh8 = oh_pool.tile([BLK, 8, BLK], f32,
                                                       tag="oh")
                                    nc.vector.tensor_tensor(
                                        out=oh8[:, :hi - ci, :],
                                        in0=dstloc[:, ci:hi].to_broadcast(
                                            [BLK, hi - ci, BLK]),
                                        in1=iota8[:, :hi - ci, :],
                                        op=OP.is_equal)
                                pos = RUN * b + BLK * k
                                q, rem = divmod(pos, CALL_IDX)
                                ensure_call(r, q)
                                nc.tensor.matmul(
                                    out=pt[:],
                                    lhsT=oh8[:, ci % 8, :],
                                    rhs=gtiles[r][q][:, rem // BLK, :],
                                    start=(r == 0 and k == 0),
                                    stop=(r == NRANGE - 1 and k == KBR - 1))
                        ev = ev_pool.tile([BLK, BLK], f32, tag="ev")
                        nc.scalar.mul(out=ev[:], in_=pt[:], mul=norm[:, b:b + 1])
                        hd = h1_dram if lnum == 0 else h2_dram
                        nc.sync.dma_start(
                            out=hd.ap()[BLK * b:BLK * (b + 1), :], in_=ev[:])
                        if lnum == 0:
                            ev2 = ev_pool.tile([BLK, BLK], f32, tag="ev2")
                            nc.scalar.mul(out=ev2[:], in_=pt[:],
                                          mul=inv[:, b:b + 1])
                            nc.sync.dma_start(
                                out=h1p_slice.ap()[BLK * b:BLK * (b + 1), :],
                                in_=ev2[:])

            layer(t0p, 0)
            nc.gpsimd.collective_compute(
                "AllGather", OP.bypass, replica_groups=RG,
                ins=[h1p_slice.ap()], outs=[t1p.ap()])
            layer(t1p, 1)

            # ======== phase 3..5 ========
            with (
                tc.tile_pool(name="fw", bufs=2) as fw,
                tc.tile_pool(name="fr", bufs=1) as fr,
                tc.tile_pool(name="fps", bufs=2, space="PSUM") as fps,
            ):
                # ---- feats3 + discrepancy row sums
                rowsums = fr.tile([BLK, BLK_PER_CORE], f32)
                t0s_v = t0s.ap().rearrange("(t p) f -> p t f", p=BLK)
                h1_v = h1_dram.ap().rearrange("(t p) f -> p t f", p=BLK)
                h2_v = h2_dram.ap().rearrange("(t p) f -> p t f", p=BLK)
                ft_v = feats_dram.ap().rearrange("(t p) f -> p t f", p=BLK)
                FG = 7  # 147 = 21 * 7
                for b0 in range(0, BLK_PER_CORE, FG):
                    ta = fw.tile([BLK, FG, 2 * D], f32, tag="fa")
                    tb = fw.tile([BLK, FG, 2 * D], f32, tag="fb")
                    tcc = fw.tile([BLK, FG, 2 * D], f32, tag="fc")
                    nc.sync.dma_start(out=ta[:], in_=t0s_v[:, b0:b0 + FG, :])
                    nc.sync.dma_start(out=tb[:], in_=h1_v[:, b0:b0 + FG, :])
                    nc.sync.dma_start(out=tcc[:], in_=h2_v[:, b0:b0 + FG, :])
                    nc.vector.tensor_add(out=ta[:], in0=ta[:], in1=tb[:])
                    nc.vector.tensor_add(out=ta[:], in0=ta[:], in1=tcc[:])
                    nc.sync.dma_start(out=ft_v[:, b0:b0 + FG, :], in_=ta[:])
                    df = fw.tile([BLK, FG, D], f32, tag="fd")
                    nc.vector.tensor_sub(out=df[:], in0=ta[:, :, 0:D],
                                         in1=ta[:, :, D:2 * D])
                    nc.vector.tensor_mul(out=df[:], in0=df[:], in1=df[:])
                    nc.vector.reduce_sum(out=rowsums[:, b0:b0 + FG], in_=df[:],
                                         axis=X)

                ind_t = fr.tile([BLK, 2, BLK_PER_CORE], f32)
                nc.sync.dma_start(out=ind_t[:],
                                  in_=ind_in.ap().rearrange("k p b -> p k b"))
                packed_d = fr.tile([BLK, 2], f32)
                wtmp = fw.tile([BLK, BLK_PER_CORE], f32, tag="wt")
                for kind in range(2):
                    nc.vector.tensor_mul(out=wtmp[:], in0=rowsums[:],
                                         in1=ind_t[:, kind, :])
                    nc.vector.reduce_sum(out=packed_d[:, kind:kind + 1],
                                         in_=wtmp[:], axis=X)
                dsc_ps = fps.tile([2, 1], f32, tag="dps")
                nc.tensor.matmul(out=dsc_ps[:], lhsT=packed_d[:],
                                 rhs=ones_col[:], start=True, stop=True)
                dsc_sb = fr.tile([2, 1], f32)
                nc.vector.tensor_copy(out=dsc_sb[:], in_=dsc_ps[:])

                # ---- zero rows_dram, write stats, gather+scatter batch rows
                zt = fw.tile([BLK, 8 * 2 * D], f32, tag="zt")
                nc.vector.memset(zt[:], 0.0)
                TGB = ROWS_N // BLK  # 96
                rows_v = rows_dram.ap()[0:TGB * BLK, :].rearrange(
                    "(t p) f -> p t f", p=BLK)
                for t0i in range(0, TGB, 8):
                    nc.sync.dma_start(
                        out=rows_v[:, t0i:t0i + 8, :],
                        in_=zt[:].rearrange("p (a f) -> p a f", a=8))
                nc.sync.dma_start(
                    out=rows_dram.ap()[TGB * BLK:ROWS_N, :],
                    in_=zt[:ROWS_N - TGB * BLK, 0:2 * D])
                # stats row cols 0:2  <- [2,1] sbuf (2 tiny descriptors)
                nc.sync.dma_start(
                    out=rows_dram.ap().rearrange("a f -> (a f)")[
                        ROW_STATS * 2 * D:ROW_STATS * 2 * D + 2],
                    in_=dsc_sb[:, 0])

                bidx_t = fr.tile([BLK, G_B // 16], i16)
                nc.sync.dma_start(out=bidx_t[:], in_=bidx_in[:, :])
                bpos_t = fr.tile([BLK, G_B // BLK], i32)
                nc.sync.dma_start(out=bpos_t[:], in_=bpos_in[:, :])
                brows = fr.tile([BLK, G_B // BLK, 2 * D], f32)
                nc.gpsimd.dma_gather(
                    out_ap=brows[:], in_ap=feats_dram.ap()[:, :],
                    idxs_ap=bidx_t[:], num_idxs=G_B, num_idxs_reg=G_B,
                    elem_size=2 * D, single_packet=False)
                nc.gpsimd.indirect_dma_start(
                    out=rows_dram.ap()[:, :],
                    out_offset=bass.IndirectOffsetOnAxis(ap=bpos_t[:], axis=0),
                    in_=brows[:],
                    in_offset=None)
                nc.gpsimd.collective_compute(
                    "AllReduce", OP.add, replica_groups=RG,
                    ins=[rows_dram.ap()], outs=[rows_full.ap()])

                # ---- final losses (every core computes the same values)
                P_t = fr.tile([BLK, BJ, 2], f32)
                N_t = fr.tile([BLK, BJ, 2], f32)
                for j in range(BJ):
                    ut = fw.tile([BLK, 2 * D], f32, tag="bu")
                    pt_ = fw.tile([BLK, 2 * D], f32, tag="bp")
                    nt = fw.tile([BLK, 2 * D], f32, tag="bn")
                    nc.sync.dma_start(out=ut[:], in_=rows_full.ap()[
                        ROLE_OFF[0] + BLK * j:ROLE_OFF[0] + BLK * (j + 1), :])
                    nc.sync.dma_start(out=pt_[:], in_=rows_full.ap()[
                        ROLE_OFF[1] + BLK * j:ROLE_OFF[1] + BLK * (j + 1), :])
                    nc.sync.dma_start(out=nt[:], in_=rows_full.ap()[
                        ROLE_OFF[2] + BLK * j:ROLE_OFF[2] + BLK * (j + 1), :])
                    nc.vector.tensor_mul(out=pt_[:], in0=ut[:], in1=pt_[:])
                    nc.vector.tensor_mul(out=nt[:], in0=ut[:], in1=nt[:])
                    nc.vector.reduce_sum(
                        out=P_t[:, j, :],
                        in_=pt_[:].rearrange("p (a f) -> p a f", a=2), axis=X)
                    nc.vector.reduce_sum(
                        out=N_t[:, j, :],
                        in_=nt[:].rearrange("p (a f) -> p a f", a=2), axis=X)
                x3i = fw.tile([BLK, BJ], f32, tag="x3i")
                x3p = fw.tile([BLK, BJ], f32, tag="x3p")
                x3t = fw.tile([BLK, BJ], f32, tag="x3t")
                nc.vector.tensor_sub(out=x3i[:], in0=P_t[:, :, 0],
                                     in1=N_t[:, :, 0])
                nc.vector.tensor_sub(out=x3p[:], in0=P_t[:, :, 1],
                                     in1=N_t[:, :, 1])
                nc.vector.tensor_add(out=x3t[:], in0=x3i[:], in1=x3p[:])
                mf = fr.tile([BLK, BJ], f32)
                nc.sync.dma_start(out=mf[:], in_=maskf_in[:, :])
                one_m = fw.tile([BLK, BJ], f32, tag="onem")
                nc.vector.tensor_scalar(out=one_m[:], in0=mf[:], scalar1=-1.0,
                                        scalar2=1.0, op0=OP.mult, op1=OP.add)
                S = 1.0 / 9.0

                def softplus(out, in_ap, scale, tag):
                    # out = ln(1 + exp(scale*in)) using Exp + Sqrt chain +
                    # one Newton step (no Ln/Softplus in the ACT tables).
                    w = fw.tile([BLK, BJ], f32, tag=tag + "w")
                    u = fw.tile([BLK, BJ], f32, tag=tag + "u")
                    sq = fw.tile([BLK, BJ], f32, tag=tag + "q")
                    nc.scalar.activation(out=w[:], in_=in_ap, func=FN.Exp,
                                         scale=scale)
                    nc.vector.tensor_scalar_add(out=w[:], in0=w[:], scalar1=1.0)
                    nc.scalar.sqrt(out=u[:], in_=w[:])
                    for _ in range(5):
                        nc.scalar.sqrt(out=u[:], in_=u[:])
                    # a = u - 1 ;  z0 = 64a - 32a^2
                    nc.vector.tensor_scalar_add(out=u[:], in0=u[:], scalar1=-1.0)
                    nc.vector.tensor_mul(out=sq[:], in0=u[:], in1=u[:])
                    nc.vector.tensor_scalar_mul(out=u[:], in0=u[:], scalar1=64.0)
                    nc.vector.tensor_scalar_mul(out=sq[:], in0=sq[:],
                                                scalar1=-32.0)
                    nc.vector.tensor_add(out=u[:], in0=u[:], in1=sq[:])
                    # newton: z1 = z0 + w*exp(-z0) - 1
                    nc.scalar.activation(out=sq[:], in_=u[:], func=FN.Exp,
                                         scale=-1.0)
                    nc.vector.tensor_mul(out=sq[:], in0=sq[:], in1=w[:])
                    nc.vector.tensor_scalar_add(out=sq[:], in0=sq[:],
                                                scalar1=-1.0)
                    nc.vector.tensor_add(out=out, in0=u[:], in1=sq[:])

                t1 = fw.tile([BLK, BJ], f32, tag="t1")
                t2 = fw.tile([BLK, BJ], f32, tag="t2")
                t3 = fw.tile([BLK, BJ], f32, tag="t3")
                t4 = fw.tile([BLK, BJ], f32, tag="t4")
                softplus(t1[:], x3i[:], -S, "a")
                softplus(t2[:], x3p[:], S, "b")
                softplus(t3[:], x3p[:], -S, "c")
                softplus(t4[:], x3t[:], -S, "d")
                nc.vector.tensor_mul(out=t1[:], in0=t1[:], in1=mf[:])
                nc.vector.tensor_mul(out=t2[:], in0=t2[:], in1=mf[:])
                nc.vector.tensor_mul(out=t3[:], in0=t3[:], in1=one_m[:])
                packed = fr.tile([BLK, 4], f32)
                nc.vector.reduce_sum(out=packed[:, 0:1], in_=t4[:], axis=X)
                nc.vector.reduce_sum(out=packed[:, 1:2], in_=t1[:], axis=X)
                nc.vector.tensor_add(out=t2[:], in0=t2[:], in1=t3[:])
                nc.vector.reduce_sum(out=packed[:, 2:3], in_=t2[:], axis=X)
                nc.vector.memset(packed[:, 3:4], 0.0)
                fin_ps = fps.tile([4, 1], f32, tag="fps")
                nc.tensor.matmul(out=fin_ps[:], lhsT=packed[:], rhs=ones_col[:],
                                 start=True, stop=True)
                cvec = fr.tile([4, 1], f32)
                nc.sync.dma_start(out=cvec[:], in_=cvec_in[:, :])
                fin = fr.tile([4, 1], f32)
                nc.scalar.mul(out=fin[:], in_=fin_ps[:], mul=cvec[:])
                # discrepancy from the AllReduced stats row
                srow = fw.tile([1, 2], f32, tag="sr")
                nc.sync.dma_start(
                    out=srow[:],
                    in_=rows_full.ap()[ROW_STATS:ROW_STATS + 1, 0:2])
                dcoef = fw.tile([1, 2], f32, tag="dc")
                nc.sync.dma_start(out=dcoef[:], in_=dcoef_in[:, :])
                nc.vector.tensor_mul(out=srow[:], in0=srow[:], in1=dcoef[:])
                dsum = fw.tile([1, 1], f32, tag="ds")
                nc.vector.reduce_sum(out=dsum[:], in_=srow[:], axis=X)
                # move dsum to partition 3 via a dram bounce, add into fin
                nc.sync.dma_start(out=scr_dram.ap()[:, :], in_=dsum[:])
                d3 = fw.tile([4, 1], f32, tag="d3")
                nc.vector.memset(d3[:], 0.0)
                nc.sync.dma_start(out=d3[3:4, :], in_=scr_dram.ap()[:, :])
                nc.vector.tensor_add(out=fin[:], in0=fin[:], in1=d3[:])
                nc.sync.dma_start(out=out4.ap(), in_=fin[:, 0])

    return nc


# ==================================================================== kernel()
_CACHE = {}


def _build_program(G_B, n_item_u, n_user_u):
    import concourse.bacc as bacc
    key = (G_B, n_item_u, n_user_u)
    if key not in _CACHE:
        nc = bacc.Bacc("TRN2", target_bir_lowering=False, debug=False,
                       num_devices=CORES)
        _build(nc, G_B, n_item_u, n_user_u)
        nc.compile()
        _CACHE[key] = nc
    return _CACHE[key]


def kernel(**inputs):
    from concourse.bass_utils import run_bass_kernel_spmd

    emb_int = np.asarray(inputs["emb_int"], np.float32)
    emb_pop = np.asarray(inputs["emb_pop"], np.float32)
    user = np.asarray(inputs["user"]).astype(np.int64)
    item_p = np.asarray(inputs["item_p"]).astype(np.int64)
    item_n = np.asarray(inputs["item_n"]).astype(np.int64)
    mask = np.asarray(inputs["mask"])
    src = np.asarray(inputs["src"]).astype(np.int64)
    dst = np.asarray(inputs["dst"]).astype(np.int64)

    t0_full = np.zeros((NPAD, 2 * D), np.float32)
    t0_full[:N, :D] = emb_int
    t0_full[:N, D:] = emb_pop

    cores_meta = [_prep_core(c, src, dst, t0_full) for c in range(CORES)]
    G_B, bidx16, bpos, ind, n_item_u, n_user_u = _prep_batch(user, item_p,
                                                             item_n)
    iota8 = np.tile(np.arange(BLK, dtype=np.float32),
                    (BLK, 8)).reshape(BLK, 8 * BLK)
    maskf = np.ascontiguousarray(
        mask.reshape(-1).astype(np.float32).reshape(BJ, BLK).T)

    nc = _build_program(G_B, n_item_u, n_user_u)

    in_maps = []
    for c in range(CORES):
        m = cores_meta[c]
        in_maps.append({
            "t0": t0_full,
            "t0s": m["t0_slice"],
            "idx16": m["idx16"],
            "dstloc": m["dstloc"],
            "deg": m["deg"],
            "ind": ind[c],
            "iota8": iota8,
            "bidx16": bidx16[c],
            "bpos": bpos[c],
            "maskf": maskf,
            "cvec": np.array([[1.0 / B], [INT_W / B], [POP_W / B], [0.0]],
                             np.float32),
            "dcoef": np.array([[-DIS_PEN / (n_item_u * D * 9.0),
                                -DIS_PEN / (n_user_u * D * 9.0)]], np.float32),
        })

    trace = bool(int(os.environ.get("LGN_TRACE", "0")))
    res = run_bass_kernel_spmd(nc, in_maps, core_ids=list(range(CORES)),
                               trace=trace)
    if trace and res.exec_time_ns is not None:
        print(f"HW exec time: {res.exec_time_ns} ns")
    return res.results[0]["out4"]


# revision 9
# speedup vs baseline: 1.6079x; 1.6079x over previous
"""LightGCN (LGN-DICE) forward loss on 8 Trainium2 NeuronCores.

Strategy (dst-sharded SpMM):
  - Pad node table to 150528 rows = 1176 blocks of 128. Core c owns dst blocks
    [147c, 147(c+1)) (rows [18816c, 18816(c+1))).
  - Fused table T0 = [emb_int | emb_pop] as [150528, 128] f32 (512B rows).
  - Each core takes the edges whose dst lies in its slice.  Edges are bucketed
    by (dst block, src range) where the 6 src ranges are overlapping 32768-row
    windows of the table (dma_gather indices are int16).  Each (block, range)
    run is padded to 5 chunks of 128 edges; each block has exactly 30 chunks
    -> fully uniform SPMD structure (one NEFF for all 8 cores; all raggedness
    lives in input data).
  - Per layer: dma_gather 512B rows of the norm-scaled table; DVE builds
    one-hots [128 edges x 128 dsts] from dst_local vs an iota row; PE matmuls
    accumulate the per-block [128, 128] aggregate in PSUM (30 matmuls/block);
    ACT evicts with per-partition norm scaling.
  - Between layers: AllGather of the norm^2-scaled layer-1 output slices
    builds the full layer-2 gather table.
  - Loss stage: feats3 = t0 + h1 + h2 (= 3*feats) per slice; discrepancy
    row-sums against host-built 0/1 indicator slices; batch rows gathered
    from the owning core's slice, scattered into a [12293, 128] buffer and
    AllReduce-summed; every core then computes the final 4 losses.
"""

import os

import numpy as np

# ---------------------------------------------------------------- constants
N_USER = 100000
N_ITEM = 50000
N = N_USER + N_ITEM          # 150000
D = 64
B = 4096
INT_W = 0.1
POP_W = 0.1
DIS_PEN = 0.01

CORES = 8
BLK = 128
BLK_PER_CORE = 147
SLICE = BLK * BLK_PER_CORE   # 18816
NPAD = SLICE * CORES         # 150528
NBLK = NPAD // BLK           # 1176

NRANGE = 6
RSTRIDE = 23552
RWIN = 32768                 # int16 index window
RBASE = [r * RSTRIDE for r in range(NRANGE)]     # last: 117760 (+32768 = 150528)

KBR = 5                      # chunks per (block, range) run
RUN = KBR * BLK              # 640 slots
CH_PER_BLK = NRANGE * KBR    # 30
CH_TOT = BLK_PER_CORE * CH_PER_BLK               # 4410 chunks per core
STREAM_L = BLK_PER_CORE * RUN                    # 94080 idx per range stream
CALL_CH = 15                 # chunks per dma_gather call
CALL_IDX = CALL_CH * BLK     # 1920
NCALL = STREAM_L // CALL_IDX                     # 49
assert NCALL * CALL_IDX == STREAM_L

PAD_DLOC = 200.0             # one-hot sentinel (no column matches)

# loss-stage row buffer: [u | ip | inn] role blocks of 4097 rows each,
# then trash row (pad scatter target) and stats row (discrepancy partials)
ROLE_OFF = [0, 4097, 2 * 4097]
ROW_TRASH = 3 * 4097         # 12291
ROW_STATS = ROW_TRASH + 1    # 12292
ROWS_N = ROW_STATS + 1       # 12293
BJ = B // BLK                # 32 batch column blocks


# =================================================================== host prep
def _wrap16(vals_i16):
    """Lay out a flat int16 idx list for dma_gather: [128, len/16] with idx i at
    (i % 16, i // 16), replicated across the 8 q7 core groups."""
    n = vals_i16.shape[0]
    assert n % 16 == 0
    w = vals_i16.reshape(n // 16, 16).T            # [16, n/16]
    return np.tile(w, (8, 1))                       # [128, n/16]


def _prep_core(c, src, dst, t0_full):
    """Build all per-core metadata arrays."""
    lo, hi = SLICE * c, SLICE * (c + 1)
    sel = (dst >= lo) & (dst < hi)
    s = src[sel].astype(np.int64)
    dl = (dst[sel] - lo).astype(np.int64)          # local dst 0..18815
    blk = dl >> 7
    # --- range assignment with rebalancing to <= RUN per (block, range).
    # Initial buckets are equal-width (25000 rows) so loads start uniform;
    # every bucket fits its 32768-row gather window: 25000(r+1)-23552r <= 32768.
    rng = np.minimum(s // (N // NRANGE), NRANGE - 1).astype(np.int64)
    for _sweep in range(6):
        cnt = np.bincount(blk * NRANGE + rng,
                          minlength=BLK_PER_CORE * NRANGE).reshape(
                              BLK_PER_CORE, NRANGE)
        over = np.argwhere(cnt > RUN)
        if len(over) == 0:
            break
        for b, r in over:
            excess = cnt[b, r] - RUN
            in_run = np.where((blk == b) & (rng == r))[0]
            for tgt in ([r - 1, r + 1] if r > 0 else [r + 1]):
                if excess <= 0 or not (0 <= tgt < NRANGE):
                    continue
                off = s[in_run] - RBASE[tgt]
                elig = in_run[(off >= 0) & (off < RWIN)]
                room = RUN - cnt[b, tgt]
                m = min(excess, len(elig), max(room, 0))
                if m > 0:
                    rng[elig[:m]] = tgt
                    cnt[b, tgt] += m
                    cnt[b, r] -= m
                    excess -= m
    cnt = np.bincount(blk * NRANGE + rng,
                      minlength=BLK_PER_CORE * NRANGE).reshape(
                          BLK_PER_CORE, NRANGE)
    assert cnt.max() <= RUN, f"core {c}: run overflow {cnt.max()} > {RUN}"

    # --- slot layout: stream r, block b occupies positions [RUN*b, RUN*(b+1))
    order = np.lexsort((dl, rng, blk))
    s, dl, rng = s[order], dl[order], rng[order]
    idx_streams = np.zeros((NRANGE, STREAM_L), np.int16)
    dstloc = np.full((BLK, CH_TOT), PAD_DLOC, np.float32)
    edge_pos = 0
    for b in range(BLK_PER_CORE):
        for r in range(NRANGE):
            nbr = cnt[b, r]
            if nbr:
                e_s = s[edge_pos:edge_pos + nbr]
                e_d = dl[edge_pos:edge_pos + nbr]
                base = RUN * b
                idx_streams[r, base:base + nbr] = (e_s - RBASE[r]).astype(np.int16)
                dloc128 = (e_d & 127).astype(np.float32)
                for k in range((nbr + BLK - 1) // BLK):
                    ci = b * CH_PER_BLK + r * KBR + k
                    seg = dloc128[k * BLK:(k + 1) * BLK]
                    dstloc[:len(seg), ci] = seg
                edge_pos += nbr
    assert edge_pos == len(s)

    idx16 = np.zeros((NRANGE, NCALL, BLK, CALL_IDX // 16), np.int16)
    for r in range(NRANGE):
        for q in range(NCALL):
            idx16[r, q] = _wrap16(idx_streams[r, q * CALL_IDX:(q + 1) * CALL_IDX])

    deg = np.bincount(dl, minlength=SLICE).astype(np.int32)
    deg_t = np.ascontiguousarray(deg.reshape(BLK_PER_CORE, BLK).T)  # [128, 147]

    return dict(idx16=idx16, dstloc=dstloc, deg=deg_t,
                t0_slice=np.ascontiguousarray(t0_full[lo:hi]))


def _prep_batch(user, item_p, item_n):
    """Batch row extraction metadata (per core) + indicator slices."""
    u = user.reshape(-1).astype(np.int64)
    ip = item_p.reshape(-1).astype(np.int64) + N_USER
    inn = item_n.reshape(-1).astype(np.int64) + N_USER
    g_all = np.concatenate([u, ip, inn])
    dest_all = np.concatenate([ROLE_OFF[k] + np.arange(B) for k in range(3)])
    owner = g_all // SLICE
    per_core = []
    for c in range(CORES):
        m = owner == c
        per_core.append((g_all[m] - SLICE * c, dest_all[m]))
    gmax = max(len(a) for a, _ in per_core)
    G_B = ((gmax + BLK - 1) // BLK) * BLK
    bidx16 = np.zeros((CORES, BLK, G_B // 16), np.int16)
    bpos = np.full((CORES, BLK, G_B // BLK), ROW_TRASH, np.int32)
    for c in range(CORES):
        loc, dest = per_core[c]
        v = np.zeros(G_B, np.int16)
        v[:len(loc)] = loc.astype(np.int16)
        bidx16[c] = _wrap16(v)
        p = np.full(G_B, ROW_TRASH, np.int32)
        p[:len(dest)] = dest
        bpos[c] = p.reshape(G_B // BLK, BLK).T
    item_nodes = np.unique(np.concatenate([ip, inn]))
    user_nodes = np.unique(u)
    ind = np.zeros((CORES, 2, BLK, BLK_PER_CORE), np.float32)
    for kind, nodes in enumerate((item_nodes, user_nodes)):
        cc = nodes // SLICE
        loc = nodes - cc * SLICE
        ind[cc, kind, loc & 127, loc >> 7] = 1.0
    return G_B, bidx16, bpos, ind, len(item_nodes), len(user_nodes)


# ============================================================== device program
def _build(nc, G_B, n_item_u, n_user_u):
    import concourse.bass as bass
    import concourse.mybir as mybir
    import concourse.tile as tile

    f32 = mybir.dt.float32
    i32 = mybir.dt.int32
    i16 = mybir.dt.int16
    FN = mybir.ActivationFunctionType
    OP = mybir.AluOpType
    X = mybir.AxisListType.X
    RG = [list(range(CORES))]

    # ---------------- I/O tensors
    t0 = nc.dram_tensor("t0", [NPAD, 2 * D], f32, kind="ExternalInput")
    t0s = nc.dram_tensor("t0s", [SLICE, 2 * D], f32, kind="ExternalInput")
    idx_in = nc.dram_tensor("idx16", [NRANGE, NCALL, BLK, CALL_IDX // 16], i16,
                            kind="ExternalInput")
    dstloc_in = nc.dram_tensor("dstloc", [BLK, CH_TOT], f32, kind="ExternalInput")
    deg_in = nc.dram_tensor("deg", [BLK, BLK_PER_CORE], i32, kind="ExternalInput")
    ind_in = nc.dram_tensor("ind", [2, BLK, BLK_PER_CORE], f32,
                            kind="ExternalInput")
    iota8_in = nc.dram_tensor("iota8", [BLK, 8 * BLK], f32, kind="ExternalInput")
    bidx_in = nc.dram_tensor("bidx16", [BLK, G_B // 16], i16, kind="ExternalInput")
    bpos_in = nc.dram_tensor("bpos", [BLK, G_B // BLK], i32, kind="ExternalInput")
    maskf_in = nc.dram_tensor("maskf", [BLK, BJ], f32, kind="ExternalInput")
    cvec_in = nc.dram_tensor("cvec", [4, 1], f32, kind="ExternalInput")
    dcoef_in = nc.dram_tensor("dcoef", [1, 2], f32, kind="ExternalInput")
    out4 = nc.dram_tensor("out4", [4], f32, kind="ExternalOutput")

    # ---------------- internal DRAM
    t0p = nc.dram_tensor("t0p", [NPAD, 2 * D], f32)        # scaled layer-1 table
    t1p = nc.dram_tensor("t1p", [NPAD, 2 * D], f32, addr_space="Shared")
    h1p_slice = nc.dram_tensor("h1p_slice", [SLICE, 2 * D], f32)
    h1_dram = nc.dram_tensor("h1_dram", [SLICE, 2 * D], f32)
    h2_dram = nc.dram_tensor("h2_dram", [SLICE, 2 * D], f32)
    feats_dram = nc.dram_tensor("feats_dram", [SLICE, 2 * D], f32)
    norm_sl_dram = nc.dram_tensor("norm_sl_dram", [SLICE], f32)
    norm_full_dram = nc.dram_tensor("norm_full_dram", [NPAD], f32,
                                    addr_space="Shared")
    rows_dram = nc.dram_tensor("rows_dram", [ROWS_N, 2 * D], f32)
    rows_full = nc.dram_tensor("rows_full", [ROWS_N, 2 * D], f32,
                               addr_space="Shared")
    scr_dram = nc.dram_tensor("scr_dram", [1, 1], f32)

    with tile.TileContext(nc) as tc:
        with tc.tile_pool(name="res", bufs=1) as res:
            # ======== phase 0: deg -> norm, AllGather norm
            ones_col = res.tile([BLK, 1], f32)
            nc.vector.memset(ones_col[:], 1.0)
            deg_i = res.tile([BLK, BLK_PER_CORE], i32)
            nc.sync.dma_start(out=deg_i[:], in_=deg_in[:, :])
            degf = res.tile([BLK, BLK_PER_CORE], f32)
            nc.vector.tensor_copy(out=degf[:], in_=deg_i[:])
            nc.vector.tensor_scalar_max(out=degf[:], in0=degf[:], scalar1=1.0)
            inv = res.tile([BLK, BLK_PER_CORE], f32)    # norm^2 = 1/max(deg,1)
            nc.vector.reciprocal(out=inv[:], in_=degf[:])
            norm = res.tile([BLK, BLK_PER_CORE], f32)
            nc.scalar.sqrt(out=norm[:], in_=inv[:])
            nc.sync.dma_start(
                out=norm_sl_dram.ap().rearrange("(b p) -> p b", p=BLK),
                in_=norm[:])
            nc.gpsimd.collective_compute(
                "AllGather", OP.bypass, replica_groups=RG,
                ins=[norm_sl_dram.ap()], outs=[norm_full_dram.ap()])
            norm_full = res.tile([BLK, NBLK], f32)
            nc.sync.dma_start(
                out=norm_full[:],
                in_=norm_full_dram.ap().rearrange("(t p) -> p t", p=BLK))

            # ======== phase 1: t0p = norm * t0 (full local table)
            GRP = 8
            t0_v = t0.ap().rearrange("(g t p) f -> g p t f", t=GRP, p=BLK)
            t0p_v = t0p.ap().rearrange("(g t p) f -> g p t f", t=GRP, p=BLK)
            with tc.tile_pool(name="sc", bufs=3) as sc:
                for g in range(NBLK // GRP):
                    tin = sc.tile([BLK, GRP, 2 * D], f32, tag="scin")
                    tout = sc.tile([BLK, GRP, 2 * D], f32, tag="scout")
                    nc.sync.dma_start(out=tin[:], in_=t0_v[g])
                    for t in range(GRP):
                        nc.scalar.mul(
                            out=tout[:, t, :], in_=tin[:, t, :],
                            mul=norm_full[:, g * GRP + t:g * GRP + t + 1])
                    nc.sync.dma_start(out=t0p_v[g], in_=tout[:])

            # ======== phase 2: the two propagation layers
            def layer(src_dram, lnum):
                with (
                    tc.tile_pool(name=f"l{lnum}cst", bufs=1) as cst,
                    tc.tile_pool(name=f"l{lnum}g0", bufs=2) as g0,
                    tc.tile_pool(name=f"l{lnum}g1", bufs=2) as g1,
                    tc.tile_pool(name=f"l{lnum}g2", bufs=2) as g2,
                    tc.tile_pool(name=f"l{lnum}g3", bufs=2) as g3,
                    tc.tile_pool(name=f"l{lnum}g4", bufs=2) as g4,
                    tc.tile_pool(name=f"l{lnum}g5", bufs=2) as g5,
                    tc.tile_pool(name=f"l{lnum}gi", bufs=4) as gidx_pool,
                    tc.tile_pool(name=f"l{lnum}oh", bufs=3) as oh_pool,
                    tc.tile_pool(name=f"l{lnum}ps", bufs=4, space="PSUM") as pp,
                    tc.tile_pool(name=f"l{lnum}ev", bufs=4) as ev_pool,
                ):
                    gpools = [g0, g1, g2, g3, g4, g5]
                    iota8 = cst.tile([BLK, 8, BLK], f32)
                    nc.sync.dma_start(
                        out=iota8[:],
                        in_=iota8_in.ap().rearrange("p (a q) -> p a q", a=8))
                    dstloc = cst.tile([BLK, CH_TOT], f32)
                    nc.sync.dma_start(out=dstloc[:], in_=dstloc_in[:, :])

                    gtiles = [[None] * NCALL for _ in range(NRANGE)]
                    qctr = [0]

                    def ensure_call(r, q):
                        if gtiles[r][q] is not None:
                            return
                        it = gidx_pool.tile([BLK, CALL_IDX // 16], i16,
                                            tag="gidx")
                        nc.sync.dma_start(out=it[:], in_=idx_in[r, q])
                        gt = gpools[r].tile([BLK, CALL_CH, BLK], f32,
                                            tag=f"g{r}")
                        nc.gpsimd.dma_gather(
                            out_ap=gt[:],
                            in_ap=src_dram.ap()[RBASE[r]:RBASE[r] + RWIN, :],
                            idxs_ap=it[:],
                            num_idxs=CALL_IDX,
                            num_idxs_reg=CALL_IDX,
                            elem_size=2 * D,
                            single_packet=False,
                            queue_num=qctr[0] % 4,
                        )
                        qctr[0] += 1
                        gtiles[r][q] = gt

                    oh8 = None
                    for b in range(BLK_PER_CORE):
                        pt = pp.tile([BLK, BLK], f32, tag="acc")
                        for r in range(NRANGE):
                            for k in range(KBR):
                                ci = b * CH_PER_BLK + r * KBR + k
                                if ci % 8 == 0:
                                    hi = min(ci + 8, CH_TOT)
                                    oh8 = oh_pool.tile([BLK, 8, BLK], f32,
                                                       tag="oh")
                                    nc.vector.tensor_tensor(
                                        out=oh8[:, :hi - ci, :],
                                        in0=dstloc[:, ci:hi].to_broadcast(
                                            [BLK, hi - ci, BLK]),
                                        in1=iota8[:, :hi - ci, :],
                                        op=OP.is_equal)
                                pos = RUN * b + BLK * k
                                q, rem = divmod(pos, CALL_IDX)
                                ensure_call(r, q)
                                nc.tensor.matmul(
                                    out=pt[:],
                                    lhsT=oh8[:, ci % 8, :],
                                    rhs=gtiles[r][q][:, rem // BLK, :],
                                    start=(r == 0 and k == 0),
                                    stop=(r == NRANGE - 1 and k == KBR - 1))
                        ev = ev_pool.tile([BLK, BLK], f32, tag="ev")
                        nc.scalar.mul(out=ev[:], in_=pt[:], mul=norm[:, b:b + 1])
                        hd = h1_dram if lnum == 0 else h2_dram
                        nc.sync.dma_start(
                            out=hd.ap()[BLK * b:BLK * (b + 1), :], in_=ev[:])
                        if lnum == 0:
                            ev2 = ev_pool.tile([BLK, BLK], f32, tag="ev2")
                            nc.scalar.mul(out=ev2[:], in_=pt[:],
                                          mul=inv[:, b:b + 1])
                            nc.sync.dma_start(
                                out=h1p_slice.ap()[BLK * b:BLK * (b + 1), :],
                                in_=ev2[:])

            layer(t0p, 0)
            nc.gpsimd.collective_compute(
                "AllGather", OP.bypass, replica_groups=RG,
                ins=[h1p_slice.ap()], outs=[t1p.ap()])
            layer(t1p, 1)

            # ======== phase 3..5 ========
            with (
                tc.tile_pool(name="fw", bufs=2) as fw,
                tc.tile_pool(name="fr", bufs=1) as fr,
                tc.tile_pool(name="fps", bufs=2, space="PSUM") as fps,
            ):
                # ---- feats3 + discrepancy row sums
                rowsums = fr.tile([BLK, BLK_PER_CORE], f32)
                t0s_v = t0s.ap().rearrange("(t p) f -> p t f", p=BLK)
                h1_v = h1_dram.ap().rearrange("(t p) f -> p t f", p=BLK)
                h2_v = h2_dram.ap().rearrange("(t p) f -> p t f", p=BLK)
                ft_v = feats_dram.ap().rearrange("(t p) f -> p t f", p=BLK)
                FG = 7  # 147 = 21 * 7
                for b0 in range(0, BLK_PER_CORE, FG):
                    ta = fw.tile([BLK, FG, 2 * D], f32, tag="fa")
                    tb = fw.tile([BLK, FG, 2 * D], f32, tag="fb")
                    tcc = fw.tile([BLK, FG, 2 * D], f32, tag="fc")
                    nc.sync.dma_start(out=ta[:], in_=t0s_v[:, b0:b0 + FG, :])
                    nc.sync.dma_start(out=tb[:], in_=h1_v[:, b0:b0 + FG, :])
                    nc.sync.dma_start(out=tcc[:], in_=h2_v[:, b0:b0 + FG, :])
                    nc.vector.tensor_add(out=ta[:], in0=ta[:], in1=tb[:])
                    nc.vector.tensor_add(out=ta[:], in0=ta[:], in1=tcc[:])
                    nc.sync.dma_start(out=ft_v[:, b0:b0 + FG, :], in_=ta[:])
                    df = fw.tile([BLK, FG, D], f32, tag="fd")
                    nc.vector.tensor_sub(out=df[:], in0=ta[:, :, 0:D],
                                         in1=ta[:, :, D:2 * D])
                    nc.vector.tensor_mul(out=df[:], in0=df[:], in1=df[:])
                    nc.vector.reduce_sum(out=rowsums[:, b0:b0 + FG], in_=df[:],
                                         axis=X)

                ind_t = fr.tile([BLK, 2, BLK_PER_CORE], f32)
                nc.sync.dma_start(out=ind_t[:],
                                  in_=ind_in.ap().rearrange("k p b -> p k b"))
                packed_d = fr.tile([BLK, 2], f32)
                wtmp = fw.tile([BLK, BLK_PER_CORE], f32, tag="wt")
                for kind in range(2):
                    nc.vector.tensor_mul(out=wtmp[:], in0=rowsums[:],
                                         in1=ind_t[:, kind, :])
                    nc.vector.reduce_sum(out=packed_d[:, kind:kind + 1],
                                         in_=wtmp[:], axis=X)
                dsc_ps = fps.tile([2, 1], f32, tag="dps")
                nc.tensor.matmul(out=dsc_ps[:], lhsT=packed_d[:],
                                 rhs=ones_col[:], start=True, stop=True)
                dsc_sb = fr.tile([2, 1], f32)
                nc.vector.tensor_copy(out=dsc_sb[:], in_=dsc_ps[:])

                # ---- zero rows_dram, write stats, gather+scatter batch rows
                zt = fw.tile([BLK, 8 * 2 * D], f32, tag="zt")
                nc.vector.memset(zt[:], 0.0)
                TGB = ROWS_N // BLK  # 96
                rows_v = rows_dram.ap()[0:TGB * BLK, :].rearrange(
                    "(t p) f -> p t f", p=BLK)
                for t0i in range(0, TGB, 8):
                    nc.sync.dma_start(
                        out=rows_v[:, t0i:t0i + 8, :],
                        in_=zt[:].rearrange("p (a f) -> p a f", a=8))
                nc.sync.dma_start(
                    out=rows_dram.ap()[TGB * BLK:ROWS_N, :],
                    in_=zt[:ROWS_N - TGB * BLK, 0:2 * D])
                # stats row cols 0:2  <- [2,1] sbuf (2 tiny descriptors)
                nc.sync.dma_start(
                    out=rows_dram.ap().rearrange("a f -> (a f)")[
                        ROW_STATS * 2 * D:ROW_STATS * 2 * D + 2],
                    in_=dsc_sb[:, 0])

                bidx_t = fr.tile([BLK, G_B // 16], i16)
                nc.sync.dma_start(out=bidx_t[:], in_=bidx_in[:, :])
                bpos_t = fr.tile([BLK, G_B // BLK], i32)
                nc.sync.dma_start(out=bpos_t[:], in_=bpos_in[:, :])
                brows = fr.tile([BLK, G_B // BLK, 2 * D], f32)
                nc.gpsimd.dma_gather(
                    out_ap=brows[:], in_ap=feats_dram.ap()[:, :],
                    idxs_ap=bidx_t[:], num_idxs=G_B, num_idxs_reg=G_B,
                    elem_size=2 * D, single_packet=False)
                nc.gpsimd.indirect_dma_start(
                    out=rows_dram.ap()[:, :],
                    out_offset=bass.IndirectOffsetOnAxis(ap=bpos_t[:], axis=0),
                    in_=brows[:],
                    in_offset=None)
                nc.gpsimd.collective_compute(
                    "AllReduce", OP.add, replica_groups=RG,
                    ins=[rows_dram.ap()], outs=[rows_full.ap()])

                # ---- final losses (every core computes the same values)
                P_t = fr.tile([BLK, BJ, 2], f32)
                N_t = fr.tile([BLK, BJ, 2], f32)
                for j in range(BJ):
                    ut = fw.tile([BLK, 2 * D], f32, tag="bu")
                    pt_ = fw.tile([BLK, 2 * D], f32, tag="bp")
                    nt = fw.tile([BLK, 2 * D], f32, tag="bn")
                    nc.sync.dma_start(out=ut[:], in_=rows_full.ap()[
                        ROLE_OFF[0] + BLK * j:ROLE_OFF[0] + BLK * (j + 1), :])
                    nc.sync.dma_start(out=pt_[:], in_=rows_full.ap()[
                        ROLE_OFF[1] + BLK * j:ROLE_OFF[1] + BLK * (j + 1), :])
                    nc.sync.dma_start(out=nt[:], in_=rows_full.ap()[
                        ROLE_OFF[2] + BLK * j:ROLE_OFF[2] + BLK * (j + 1), :])
                    nc.vector.tensor_mul(out=pt_[:], in0=ut[:], in1=pt_[:])
                    nc.vector.tensor_mul(out=nt[:], in0=ut[:], in1=nt[:])
                    nc.vector.reduce_sum(
                        out=P_t[:, j, :],
                        in_=pt_[:].rearrange("p (a f) -> p a f", a=2), axis=X)
                    nc.vector.reduce_sum(
                        out=N_t[:, j, :],
                        in_=nt[:].rearrange("p (a f) -> p a f", a=2), axis=X)
                x3i = fw.tile([BLK, BJ], f32, tag="x3i")
                x3p = fw.tile([BLK, BJ], f32, tag="x3p")
                x3t = fw.tile([BLK, BJ], f32, tag="x3t")
                nc.vector.tensor_sub(out=x3i[:], in0=P_t[:, :, 0],
                                     in1=N_t[:, :, 0])
                nc.vector.tensor_sub(out=x3p[:], in0=P_t[:, :, 1],
                                     in1=N_t[:, :, 1])
                nc.vector.tensor_add(out=x3t[:], in0=x3i[:], in1=x3p[:])
                mf = fr.tile([BLK, BJ], f32)
                nc.sync.dma_start(out=mf[:], in_=maskf_in[:, :])
                one_m = fw.tile([BLK, BJ], f32, tag="onem")
                nc.vector.tensor_scalar(out=one_m[:], in0=mf[:], scalar1=-1.0,
                                        scalar2=1.0, op0=OP.mult, op1=OP.add)
                S = 1.0 / 9.0

                def softplus(out, in_ap, scale, tag):
                    # out = ln(1 + exp(scale*in)) using Exp + Sqrt chain +
                    # one Newton step (no Ln/Softplus in the ACT tables).
                    w = fw.tile([BLK, BJ], f32, tag=tag + "w")
                    u = fw.tile([BLK, BJ], f32, tag=tag + "u")
                    sq = fw.tile([BLK, BJ], f32, tag=tag + "q")
                    nc.scalar.activation(out=w[:], in_=in_ap, func=FN.Exp,
                                         scale=scale)
                    nc.vector.tensor_scalar_add(out=w[:], in0=w[:], scalar1=1.0)
                    nc.scalar.sqrt(out=u[:], in_=w[:])
                    for _ in range(5):
                        nc.scalar.sqrt(out=u[:], in_=u[:])
                    # a = u - 1 ;  z0 = 64a - 32a^2
                    nc.vector.tensor_scalar_add(out=u[:], in0=u[:], scalar1=-1.0)
                    nc.vector.tensor_mul(out=sq[:], in0=u[:], in1=u[:])
                    nc.vector.tensor_scalar_mul(out=u[:], in0=u[:], scalar1=64.0)
                    nc.vector.tensor_scalar_mul(out=sq[:], in0=sq[:],
                                                scalar1=-32.0)
                    nc.vector.tensor_add(out=u[:], in0=u[:], in1=sq[:])
                    # newton: z1 = z0 + w*exp(-z0) - 1
                    nc.scalar.activation(out=sq[:], in_=u[:], func=FN.Exp,
                                         scale=-1.0)
                    nc.vector.tensor_mul(out=sq[:], in0=sq[:], in1=w[:])
                    nc.vector.tensor_scalar_add(out=sq[:], in0=sq[:],
                                                scalar1=-1.0)
                    nc.vector.tensor_add(out=out, in0=u[:], in1=sq[:])

                t1 = fw.tile([BLK, BJ], f32, tag="t1")
                t2 = fw.tile([BLK, BJ], f32, tag="t2")
                t3 = fw.tile([BLK, BJ], f32, tag="t3")
                t4 = fw.tile([BLK, BJ], f32, tag="t4")
                softplus(t1[:], x3i[:], -S, "a")
                softplus(t2[:], x3p[:], S, "b")
                softplus(t3[:], x3p[:], -S, "c")
                softplus(t4[:], x3t[:], -S, "d")
                nc.vector.tensor_mul(out=t1[:], in0=t1[:], in1=mf[:])
                nc.vector.tensor_mul(out=t2[:], in0=t2[:], in1=mf[:])
                nc.vector.tensor_mul(out=t3[:], in0=t3[:], in1=one_m[:])
                packed = fr.tile([BLK, 4], f32)
                nc.vector.reduce_sum(out=packed[:, 0:1], in_=t4[:], axis=X)
                nc.vector.reduce_sum(out=packed[:, 1:2], in_=t1[:], axis=X)
                nc.vector.tensor_add(out=t2[:], in0=t2[:], in1=t3[:])
                nc.vector.reduce_sum(out=packed[:, 2:3], in_=t2[:], axis=X)
                nc.vector.memset(packed[:, 3:4], 0.0)
                fin_ps = fps.tile([4, 1], f32, tag="fps")
                nc.tensor.matmul(out=fin_ps[:], lhsT=packed[:], rhs=ones_col[:],
                                 start=True, stop=True)
                cvec = fr.tile([4, 1], f32)
                nc.sync.dma_start(out=cvec[:], in_=cvec_in[:, :])
                fin = fr.tile([4, 1], f32)
                nc.scalar.mul(out=fin[:], in_=fin_ps[:], mul=cvec[:])
                # discrepancy from the AllReduced stats row
                srow = fw.tile([1, 2], f32, tag="sr")
                nc.sync.dma_start(
                    out=srow[:],
                    in_=rows_full.ap()[ROW_STATS:ROW_STATS + 1, 0:2])
                dcoef = fw.tile([1, 2], f32, tag="dc")
                nc.sync.dma_start(out=dcoef[:], in_=dcoef_in[:, :])
                nc.vector.tensor_mul(out=srow[:], in0=srow[:], in1=dcoef[:])
                dsum = fw.tile([1, 1], f32, tag="ds")
                nc.vector.reduce_sum(out=dsum[:], in_=srow[:], axis=X)
                # move dsum to partition 3 via a dram bounce, add into fin
                nc.sync.dma_start(out=scr_dram.ap()[:, :], in_=dsum[:])
                d3 = fw.tile([4, 1], f32, tag="d3")
                nc.vector.memset(d3[:], 0.0)
                nc.sync.dma_start(out=d3[3:4, :], in_=scr_dram.ap()[:, :])
                nc.vector.tensor_add(out=fin[:], in0=fin[:], in1=d3[:])
                nc.sync.dma_start(out=out4.ap(), in_=fin[:, 0])

    return nc


# ==================================================================== kernel()
_CACHE = {}


def _build_program(G_B, n_item_u, n_user_u):
    import concourse.bacc as bacc
    key = (G_B, n_item_u, n_user_u)
    if key not in _CACHE:
        nc = bacc.Bacc("TRN2", target_bir_lowering=False, debug=False,
                       num_devices=CORES, num_swdge_queues=4)
        _build(nc, G_B, n_item_u, n_user_u)
        nc.compile()
        _CACHE[key] = nc
    return _CACHE[key]


def kernel(**inputs):
    from concourse.bass_utils import run_bass_kernel_spmd

    emb_int = np.asarray(inputs["emb_int"], np.float32)
    emb_pop = np.asarray(inputs["emb_pop"], np.float32)
    user = np.asarray(inputs["user"]).astype(np.int64)
    item_p = np.asarray(inputs["item_p"]).astype(np.int64)
    item_n = np.asarray(inputs["item_n"]).astype(np.int64)
    mask = np.asarray(inputs["mask"])
    src = np.asarray(inputs["src"]).astype(np.int64)
    dst = np.asarray(inputs["dst"]).astype(np.int64)

    t0_full = np.zeros((NPAD, 2 * D), np.float32)
    t0_full[:N, :D] = emb_int
    t0_full[:N, D:] = emb_pop

    cores_meta = [_prep_core(c, src, dst, t0_full) for c in range(CORES)]
    G_B, bidx16, bpos, ind, n_item_u, n_user_u = _prep_batch(user, item_p,
                                                             item_n)
    iota8 = np.tile(np.arange(BLK, dtype=np.float32),
                    (BLK, 8)).reshape(BLK, 8 * BLK)
    maskf = np.ascontiguousarray(
        mask.reshape(-1).astype(np.float32).reshape(BJ, BLK).T)

    nc = _build_program(G_B, n_item_u, n_user_u)

    in_maps = []
    for c in range(CORES):
        m = cores_meta[c]
        in_maps.append({
            "t0": t0_full,
            "t0s": m["t0_slice"],
            "idx16": m["idx16"],
            "dstloc": m["dstloc"],
            "deg": m["deg"],
            "ind": ind[c],
            "iota8": iota8,
            "bidx16": bidx16[c],
            "bpos": bpos[c],
            "maskf": maskf,
            "cvec": np.array([[1.0 / B], [INT_W / B], [POP_W / B], [0.0]],
                             np.float32),
            "dcoef": np.array([[-DIS_PEN / (n_item_u * D * 9.0),
                                -DIS_PEN / (n_user_u * D * 9.0)]], np.float32),
        })

    trace = bool(int(os.environ.get("LGN_TRACE", "0")))
    res = run_bass_kernel_spmd(nc, in_maps, core_ids=list(range(CORES)),
                               trace=trace)
    if trace and res.exec_time_ns is not None:
        print(f"HW exec time: {res.exec_time_ns} ns")
    return res.results[0]["out4"]


# revision 16
# speedup vs baseline: 1.8989x; 1.1809x over previous
"""LightGCN (LGN-DICE) forward loss on 8 Trainium2 NeuronCores.

Strategy (dst-sharded SpMM):
  - Pad node table to 150528 rows = 1176 blocks of 128. Core c owns dst blocks
    [147c, 147(c+1)) (rows [18816c, 18816(c+1))).
  - Fused table T0 = [emb_int | emb_pop] as [150528, 128] f32 (512B rows).
  - Each core takes the edges whose dst lies in its slice.  Edges are bucketed
    by (dst block, src range) where the 6 src ranges are overlapping 32768-row
    windows of the table (dma_gather indices are int16).  Each (block, range)
    run is padded to 5 chunks of 128 edges; each block has exactly 30 chunks
    -> fully uniform SPMD structure (one NEFF for all 8 cores; all raggedness
    lives in input data).
  - Per layer: dma_gather 512B rows of the norm-scaled table; DVE builds
    one-hots [128 edges x 128 dsts] from dst_local vs an iota row; PE matmuls
    accumulate the per-block [128, 128] aggregate in PSUM (30 matmuls/block);
    ACT evicts with per-partition norm scaling.
  - Between layers: AllGather of the norm^2-scaled layer-1 output slices
    builds the full layer-2 gather table.
  - Loss stage: feats3 = t0 + h1 + h2 (= 3*feats) per slice; discrepancy
    row-sums against host-built 0/1 indicator slices; batch rows gathered
    from the owning core's slice, scattered into a [12293, 128] buffer and
    AllReduce-summed; every core then computes the final 4 losses.
"""

import os

import numpy as np

# ---------------------------------------------------------------- constants
N_USER = 100000
N_ITEM = 50000
N = N_USER + N_ITEM          # 150000
D = 64
B = 4096
INT_W = 0.1
POP_W = 0.1
DIS_PEN = 0.01

CORES = 8
BLK = 128
BLK_PER_CORE = 147
SLICE = BLK * BLK_PER_CORE   # 18816
NPAD = SLICE * CORES         # 150528
NBLK = NPAD // BLK           # 1176

NRANGE = 6
RSTRIDE = 23552
RWIN = 32768                 # int16 index window
RBASE = [r * RSTRIDE for r in range(NRANGE)]     # last: 117760 (+32768 = 150528)

KBR = 5                      # chunks per (block, range) run
RUN = KBR * BLK              # 640 slots
CH_PER_BLK = NRANGE * KBR    # 30
CH_TOT = BLK_PER_CORE * CH_PER_BLK               # 4410 chunks per core
STREAM_L = BLK_PER_CORE * RUN                    # 94080 idx per range stream
CALL_CH = 15                 # chunks per dma_gather call
CALL_IDX = CALL_CH * BLK     # 1920
NCALL = STREAM_L // CALL_IDX                     # 49
assert NCALL * CALL_IDX == STREAM_L

PAD_DLOC = 200.0             # one-hot sentinel (no column matches)

# loss-stage row buffer: [u | ip | inn] role blocks of 4096 rows each,
# then a stats row (discrepancy partials).  Every core gathers ALL 12288
# batch slots from its slice (garbage rows for slots it does not own), masks
# non-owned slots to zero on DVE, writes contiguously, and the AllReduce
# assembles the full buffer -- no indirect scatter involved.
ROLE_OFF = [0, B, 2 * B]
ROW_STATS = 3 * B            # 12288
ROWS_N = ROW_STATS + 1       # 12289
GB_ROWS = 3 * B              # gathered batch slots per core
BJ = B // BLK                # 32 batch column blocks
NQ = 4                       # SWDGE gather queues


# =================================================================== host prep
def _wrap16(vals_i16):
    """Lay out a flat int16 idx list for dma_gather: [128, len/16] with idx i at
    (i % 16, i // 16), replicated across the 8 q7 core groups."""
    n = vals_i16.shape[0]
    assert n % 16 == 0
    w = vals_i16.reshape(n // 16, 16).T            # [16, n/16]
    return np.tile(w, (8, 1))                       # [128, n/16]


def _prep_core(c, src, dst, t0_full):
    """Build all per-core metadata arrays."""
    lo, hi = SLICE * c, SLICE * (c + 1)
    sel = (dst >= lo) & (dst < hi)
    s = src[sel].astype(np.int64)
    dl = (dst[sel] - lo).astype(np.int64)          # local dst 0..18815
    blk = dl >> 7
    # --- range assignment with rebalancing to <= RUN per (block, range).
    # Initial buckets are equal-width (25000 rows) so loads start uniform;
    # every bucket fits its 32768-row gather window: 25000(r+1)-23552r <= 32768.
    rng = np.minimum(s // (N // NRANGE), NRANGE - 1).astype(np.int64)
    for _sweep in range(6):
        cnt = np.bincount(blk * NRANGE + rng,
                          minlength=BLK_PER_CORE * NRANGE).reshape(
                              BLK_PER_CORE, NRANGE)
        over = np.argwhere(cnt > RUN)
        if len(over) == 0:
            break
        for b, r in over:
            excess = cnt[b, r] - RUN
            in_run = np.where((blk == b) & (rng == r))[0]
            for tgt in ([r - 1, r + 1] if r > 0 else [r + 1]):
                if excess <= 0 or not (0 <= tgt < NRANGE):
                    continue
                off = s[in_run] - RBASE[tgt]
                elig = in_run[(off >= 0) & (off < RWIN)]
                room = RUN - cnt[b, tgt]
                m = min(excess, len(elig), max(room, 0))
                if m > 0:
                    rng[elig[:m]] = tgt
                    cnt[b, tgt] += m
                    cnt[b, r] -= m
                    excess -= m
    cnt = np.bincount(blk * NRANGE + rng,
                      minlength=BLK_PER_CORE * NRANGE).reshape(
                          BLK_PER_CORE, NRANGE)
    assert cnt.max() <= RUN, f"core {c}: run overflow {cnt.max()} > {RUN}"

    # --- slot layout: stream r, block b occupies positions [RUN*b, RUN*(b+1))
    order = np.lexsort((dl, rng, blk))
    s, dl, rng = s[order], dl[order], rng[order]
    idx_streams = np.zeros((NRANGE, STREAM_L), np.int16)
    dstloc = np.full((BLK, CH_TOT), PAD_DLOC, np.float32)
    edge_pos = 0
    for b in range(BLK_PER_CORE):
        for r in range(NRANGE):
            nbr = cnt[b, r]
            if nbr:
                e_s = s[edge_pos:edge_pos + nbr]
                e_d = dl[edge_pos:edge_pos + nbr]
                base = RUN * b
                idx_streams[r, base:base + nbr] = (e_s - RBASE[r]).astype(np.int16)
                dloc128 = (e_d & 127).astype(np.float32)
                for k in range((nbr + BLK - 1) // BLK):
                    ci = b * CH_PER_BLK + r * KBR + k
                    seg = dloc128[k * BLK:(k + 1) * BLK]
                    dstloc[:len(seg), ci] = seg
                edge_pos += nbr
    assert edge_pos == len(s)

    idx16 = np.zeros((NRANGE, NCALL, BLK, CALL_IDX // 16), np.int16)
    for r in range(NRANGE):
        for q in range(NCALL):
            idx16[r, q] = _wrap16(idx_streams[r, q * CALL_IDX:(q + 1) * CALL_IDX])

    deg = np.bincount(dl, minlength=SLICE).astype(np.int32)
    deg_t = np.ascontiguousarray(deg.reshape(BLK_PER_CORE, BLK).T)  # [128, 147]

    return dict(idx16=idx16, dstloc=dstloc, deg=deg_t,
                t0_slice=np.ascontiguousarray(t0_full[lo:hi]))


def _prep_batch(user, item_p, item_n):
    """Batch row extraction metadata (per core) + indicator slices."""
    u = user.reshape(-1).astype(np.int64)
    ip = item_p.reshape(-1).astype(np.int64) + N_USER
    inn = item_n.reshape(-1).astype(np.int64) + N_USER
    g_all = np.concatenate([u, ip, inn])          # slot s = role*B + b
    owner = g_all // SLICE
    loc = g_all - owner * SLICE
    bidx16 = np.zeros((CORES, BLK, GB_ROWS // 16), np.int16)
    ownmask = np.zeros((CORES, BLK, GB_ROWS // BLK), np.float32)
    for c in range(CORES):
        m = owner == c
        v = np.where(m, loc, 0).astype(np.int16)
        bidx16[c] = _wrap16(v)
        # gathered slot s lands at [s % 128, s // 128]
        ownmask[c] = m.astype(np.float32).reshape(GB_ROWS // BLK, BLK).T
    item_nodes = np.unique(np.concatenate([ip, inn]))
    user_nodes = np.unique(u)
    ind = np.zeros((CORES, 2, BLK, BLK_PER_CORE), np.float32)
    for kind, nodes in enumerate((item_nodes, user_nodes)):
        cc = nodes // SLICE
        l2 = nodes - cc * SLICE
        ind[cc, kind, l2 & 127, l2 >> 7] = 1.0
    return bidx16, ownmask, ind, len(item_nodes), len(user_nodes)


# ============================================================== device program
def _build(nc, n_item_u, n_user_u):
    import concourse.bass as bass
    import concourse.mybir as mybir
    import concourse.tile as tile

    f32 = mybir.dt.float32
    bf16 = mybir.dt.bfloat16
    i32 = mybir.dt.int32
    i16 = mybir.dt.int16
    FN = mybir.ActivationFunctionType
    OP = mybir.AluOpType
    X = mybir.AxisListType.X
    RG = [list(range(CORES))]

    # ---------------- I/O tensors
    t0 = nc.dram_tensor("t0", [NPAD, 2 * D], f32, kind="ExternalInput")
    t0s = nc.dram_tensor("t0s", [SLICE, 2 * D], f32, kind="ExternalInput")
    idx_in = nc.dram_tensor("idx16", [NRANGE, NCALL, BLK, CALL_IDX // 16], i16,
                            kind="ExternalInput")
    dstloc_in = nc.dram_tensor("dstloc", [BLK, CH_TOT], bf16,
                               kind="ExternalInput")
    deg_in = nc.dram_tensor("deg", [BLK, BLK_PER_CORE], i32, kind="ExternalInput")
    ind_in = nc.dram_tensor("ind", [2, BLK, BLK_PER_CORE], f32,
                            kind="ExternalInput")
    iota8_in = nc.dram_tensor("iota8", [BLK, 8 * BLK], bf16,
                              kind="ExternalInput")
    bidx_in = nc.dram_tensor("bidx16", [BLK, GB_ROWS // 16], i16,
                             kind="ExternalInput")
    own_in = nc.dram_tensor("ownmask", [BLK, GB_ROWS // BLK], f32,
                            kind="ExternalInput")
    maskf_in = nc.dram_tensor("maskf", [BLK, BJ], f32, kind="ExternalInput")
    cvec_in = nc.dram_tensor("cvec", [4, 1], f32, kind="ExternalInput")
    dcoef_in = nc.dram_tensor("dcoef", [1, 2], f32, kind="ExternalInput")
    out4 = nc.dram_tensor("out4", [4], f32, kind="ExternalOutput")
    debug = bool(int(os.environ.get("LGN_DEBUG", "0")))
    if debug:
        dbg_pre = nc.dram_tensor("dbg_pre", [ROWS_N, 2 * D], f32,
                                 kind="ExternalOutput")
        dbg_post = nc.dram_tensor("dbg_post", [ROWS_N, 2 * D], f32,
                                  kind="ExternalOutput")
        dbg_feats = nc.dram_tensor("dbg_feats", [SLICE, 2 * D], f32,
                                   kind="ExternalOutput")


    # ---------------- internal DRAM
    t0p = nc.dram_tensor("t0p", [NPAD, 2 * D], bf16)       # scaled layer-1 table
    t1p = nc.dram_tensor("t1p", [NPAD, 2 * D], bf16, addr_space="Shared")
    h1p_slice = nc.dram_tensor("h1p_slice", [SLICE, 2 * D], bf16)
    h1_dram = nc.dram_tensor("h1_dram", [SLICE, 2 * D], f32)
    h2_dram = nc.dram_tensor("h2_dram", [SLICE, 2 * D], f32)
    feats_dram = nc.dram_tensor("feats_dram", [SLICE, 2 * D], f32)
    norm_sl_dram = nc.dram_tensor("norm_sl_dram", [SLICE], f32)
    norm_full_dram = nc.dram_tensor("norm_full_dram", [NPAD], f32,
                                    addr_space="Shared")
    rows_dram = nc.dram_tensor("rows_dram", [ROWS_N, 2 * D], f32)
    rows_full = nc.dram_tensor("rows_full", [ROWS_N, 2 * D], f32,
                               addr_space="Shared")
    scr_dram = nc.dram_tensor("scr_dram", [1, 1], f32)

    with tile.TileContext(nc) as tc:
        with tc.tile_pool(name="res", bufs=1) as res:
            # ======== phase 0: deg -> norm, AllGather norm
            ones_col = res.tile([BLK, 1], f32)
            nc.vector.memset(ones_col[:], 1.0)
            deg_i = res.tile([BLK, BLK_PER_CORE], i32)
            nc.sync.dma_start(out=deg_i[:], in_=deg_in[:, :])
            degf = res.tile([BLK, BLK_PER_CORE], f32)
            nc.vector.tensor_copy(out=degf[:], in_=deg_i[:])
            nc.vector.tensor_scalar_max(out=degf[:], in0=degf[:], scalar1=1.0)
            inv = res.tile([BLK, BLK_PER_CORE], f32)    # norm^2 = 1/max(deg,1)
            nc.vector.reciprocal(out=inv[:], in_=degf[:])
            norm = res.tile([BLK, BLK_PER_CORE], f32)
            nc.scalar.sqrt(out=norm[:], in_=inv[:])
            nc.sync.dma_start(
                out=norm_sl_dram.ap().rearrange("(b p) -> p b", p=BLK),
                in_=norm[:])
            nc.gpsimd.collective_compute(
                "AllGather", OP.bypass, replica_groups=RG,
                ins=[norm_sl_dram.ap()], outs=[norm_full_dram.ap()])
            norm_full = res.tile([BLK, NBLK], f32)
            nc.sync.dma_start(
                out=norm_full[:],
                in_=norm_full_dram.ap().rearrange("(t p) -> p t", p=BLK))

            # ======== phase 1: t0p = norm * t0 (full local table)
            GRP = 8
            t0_v = t0.ap().rearrange("(g t p) f -> g p t f", t=GRP, p=BLK)
            t0p_v = t0p.ap().rearrange("(g t p) f -> g p t f", t=GRP, p=BLK)
            with tc.tile_pool(name="sc", bufs=3) as sc:
                for g in range(NBLK // GRP):
                    tin = sc.tile([BLK, GRP, 2 * D], f32, tag="scin")
                    tout = sc.tile([BLK, GRP, 2 * D], bf16, tag="scout")
                    nc.sync.dma_start(out=tin[:], in_=t0_v[g])
                    for t in range(GRP):
                        nc.scalar.mul(
                            out=tout[:, t, :], in_=tin[:, t, :],
                            mul=norm_full[:, g * GRP + t:g * GRP + t + 1])
                    nc.sync.dma_start(out=t0p_v[g], in_=tout[:])

            # ======== phase 2: the two propagation layers
            def layer(src_dram, lnum):
                with (
                    tc.tile_pool(name=f"l{lnum}cst", bufs=1) as cst,
                    tc.tile_pool(name=f"l{lnum}g0", bufs=2) as g0,
                    tc.tile_pool(name=f"l{lnum}g1", bufs=2) as g1,
                    tc.tile_pool(name=f"l{lnum}g2", bufs=2) as g2,
                    tc.tile_pool(name=f"l{lnum}g3", bufs=2) as g3,
                    tc.tile_pool(name=f"l{lnum}g4", bufs=2) as g4,
                    tc.tile_pool(name=f"l{lnum}g5", bufs=2) as g5,
                    tc.tile_pool(name=f"l{lnum}gi", bufs=4) as gidx_pool,
                    tc.tile_pool(name=f"l{lnum}oh", bufs=3) as oh_pool,
                    tc.tile_pool(name=f"l{lnum}ps", bufs=4, space="PSUM") as pp,
                    tc.tile_pool(name=f"l{lnum}ev", bufs=4) as ev_pool,
                ):
                    gpools = [g0, g1, g2, g3, g4, g5]
                    iota8 = cst.tile([BLK, 8, BLK], bf16)
                    nc.sync.dma_start(
                        out=iota8[:],
                        in_=iota8_in.ap().rearrange("p (a q) -> p a q", a=8))
                    dstloc = cst.tile([BLK, CH_TOT], bf16)
                    nc.sync.dma_start(out=dstloc[:], in_=dstloc_in[:, :])

                    gtiles = [[None] * NCALL for _ in range(NRANGE)]
                    qctr = [0]

                    def ensure_call(r, q):
                        if gtiles[r][q] is not None:
                            return
                        it = gidx_pool.tile([BLK, CALL_IDX // 16], i16,
                                            tag="gidx")
                        nc.sync.dma_start(out=it[:], in_=idx_in[r, q])
                        gt = gpools[r].tile([BLK, CALL_CH, BLK], bf16,
                                            tag=f"g{r}")
                        nc.gpsimd.dma_gather(
                            out_ap=gt[:],
                            in_ap=src_dram.ap()[RBASE[r]:RBASE[r] + RWIN, :],
                            idxs_ap=it[:],
                            num_idxs=CALL_IDX,
                            num_idxs_reg=CALL_IDX,
                            elem_size=2 * D,
                            single_packet=False,
                            queue_num=qctr[0] % NQ,
                        )
                        qctr[0] += 1
                        gtiles[r][q] = gt

                    oh8 = None
                    for b in range(BLK_PER_CORE):
                        pt = pp.tile([BLK, BLK], f32, tag="acc")
                        for r in range(NRANGE):
                            for k in range(KBR):
                                ci = b * CH_PER_BLK + r * KBR + k
                                if ci % 8 == 0:
                                    hi = min(ci + 8, CH_TOT)
                                    oh8 = oh_pool.tile([BLK, 8, BLK], bf16,
                                                       tag="oh")
                                    nc.vector.tensor_tensor(
                                        out=oh8[:, :hi - ci, :],
                                        in0=dstloc[:, ci:hi].to_broadcast(
                                            [BLK, hi - ci, BLK]),
                                        in1=iota8[:, :hi - ci, :],
                                        op=OP.is_equal)
                                pos = RUN * b + BLK * k
                                q, rem = divmod(pos, CALL_IDX)
                                ensure_call(r, q)
                                nc.tensor.matmul(
                                    out=pt[:],
                                    lhsT=oh8[:, ci % 8, :],
                                    rhs=gtiles[r][q][:, rem // BLK, :],
                                    start=(r == 0 and k == 0),
                                    stop=(r == NRANGE - 1 and k == KBR - 1))
                        ev = ev_pool.tile([BLK, BLK], f32, tag="ev")
                        nc.scalar.mul(out=ev[:], in_=pt[:], mul=norm[:, b:b + 1])
                        hd = h1_dram if lnum == 0 else h2_dram
                        nc.sync.dma_start(
                            out=hd.ap()[BLK * b:BLK * (b + 1), :], in_=ev[:])
                        if lnum == 0:
                            ev2 = ev_pool.tile([BLK, BLK], bf16, tag="ev2")
                            nc.scalar.mul(out=ev2[:], in_=pt[:],
                                          mul=inv[:, b:b + 1])
                            nc.sync.dma_start(
                                out=h1p_slice.ap()[BLK * b:BLK * (b + 1), :],
                                in_=ev2[:])

            layer(t0p, 0)
            nc.gpsimd.collective_compute(
                "AllGather", OP.bypass, replica_groups=RG,
                ins=[h1p_slice.ap()], outs=[t1p.ap()])
            layer(t1p, 1)

            # ======== phase 3..5 ========
            with (
                tc.tile_pool(name="fw", bufs=2) as fw,
                tc.tile_pool(name="fr", bufs=1) as fr,
                tc.tile_pool(name="fps", bufs=2, space="PSUM") as fps,
            ):
                # ---- feats3 + discrepancy row sums
                rowsums = fr.tile([BLK, BLK_PER_CORE], f32)
                t0s_v = t0s.ap().rearrange("(t p) f -> p t f", p=BLK)
                h1_v = h1_dram.ap().rearrange("(t p) f -> p t f", p=BLK)
                h2_v = h2_dram.ap().rearrange("(t p) f -> p t f", p=BLK)
                ft_v = feats_dram.ap().rearrange("(t p) f -> p t f", p=BLK)
                FG = 7  # 147 = 21 * 7
                for b0 in range(0, BLK_PER_CORE, FG):
                    ta = fw.tile([BLK, FG, 2 * D], f32, tag="fa")
                    tb = fw.tile([BLK, FG, 2 * D], f32, tag="fb")
                    tcc = fw.tile([BLK, FG, 2 * D], f32, tag="fc")
                    nc.sync.dma_start(out=ta[:], in_=t0s_v[:, b0:b0 + FG, :])
                    nc.sync.dma_start(out=tb[:], in_=h1_v[:, b0:b0 + FG, :])
                    nc.sync.dma_start(out=tcc[:], in_=h2_v[:, b0:b0 + FG, :])
                    nc.vector.tensor_add(out=ta[:], in0=ta[:], in1=tb[:])
                    nc.vector.tensor_add(out=ta[:], in0=ta[:], in1=tcc[:])
                    nc.sync.dma_start(out=ft_v[:, b0:b0 + FG, :], in_=ta[:])
                    df = fw.tile([BLK, FG, D], f32, tag="fd")
                    nc.vector.tensor_sub(out=df[:], in0=ta[:, :, 0:D],
                                         in1=ta[:, :, D:2 * D])
                    nc.vector.tensor_mul(out=df[:], in0=df[:], in1=df[:])
                    nc.vector.reduce_sum(out=rowsums[:, b0:b0 + FG], in_=df[:],
                                         axis=X)

                ind_t = fr.tile([BLK, 2, BLK_PER_CORE], f32)
                nc.sync.dma_start(out=ind_t[:],
                                  in_=ind_in.ap().rearrange("k p b -> p k b"))
                packed_d = fr.tile([BLK, 2], f32)
                wtmp = fw.tile([BLK, BLK_PER_CORE], f32, tag="wt")
                for kind in range(2):
                    nc.vector.tensor_mul(out=wtmp[:], in0=rowsums[:],
                                         in1=ind_t[:, kind, :])
                    nc.vector.reduce_sum(out=packed_d[:, kind:kind + 1],
                                         in_=wtmp[:], axis=X)
                dsc_ps = fps.tile([2, 1], f32, tag="dps")
                nc.tensor.matmul(out=dsc_ps[:], lhsT=packed_d[:],
                                 rhs=ones_col[:], start=True, stop=True)
                dsc_sb = fr.tile([2, 1], f32)
                nc.vector.tensor_copy(out=dsc_sb[:], in_=dsc_ps[:])

                # ---- stats row, then gather-all-slots + ownership mask
                zt = fw.tile([1, 2 * D], f32, tag="zt")
                nc.vector.memset(zt[:], 0.0)
                nc.sync.dma_start(
                    out=rows_dram.ap()[ROW_STATS:ROW_STATS + 1, :], in_=zt[:])
                nc.sync.dma_start(
                    out=rows_dram.ap().rearrange("a f -> (a f)")[
                        ROW_STATS * 2 * D:ROW_STATS * 2 * D + 2],
                    in_=dsc_sb[:, 0])

                bidx_t = fr.tile([BLK, GB_ROWS // 16], i16)
                nc.sync.dma_start(out=bidx_t[:], in_=bidx_in[:, :])
                own_t = fr.tile([BLK, GB_ROWS // BLK], f32)
                nc.sync.dma_start(out=own_t[:], in_=own_in[:, :])
                brows = fr.tile([BLK, GB_ROWS // BLK, 2 * D], f32)
                nc.gpsimd.dma_gather(
                    out_ap=brows[:], in_ap=feats_dram.ap()[:, :],
                    idxs_ap=bidx_t[:], num_idxs=GB_ROWS, num_idxs_reg=GB_ROWS,
                    elem_size=2 * D, single_packet=False)
                nc.vector.tensor_tensor(
                    out=brows[:], in0=brows[:],
                    in1=own_t[:].to_broadcast([BLK, GB_ROWS // BLK, 2 * D]),
                    op=OP.mult)
                nc.sync.dma_start(
                    out=rows_dram.ap()[0:GB_ROWS, :].rearrange(
                        "(t p) f -> p t f", p=BLK),
                    in_=brows[:])
                nc.gpsimd.collective_compute(
                    "AllReduce", OP.add, replica_groups=RG,
                    ins=[rows_dram.ap()], outs=[rows_full.ap()])
                if debug:
                    nc.sync.dma_start(out=dbg_post.ap()[:, :],
                                      in_=rows_full.ap()[:, :])

                # ---- final losses (every core computes the same values)
                P_t = fr.tile([BLK, BJ, 2], f32)
                N_t = fr.tile([BLK, BJ, 2], f32)
                for j in range(BJ):
                    ut = fw.tile([BLK, 2 * D], f32, tag="bu")
                    pt_ = fw.tile([BLK, 2 * D], f32, tag="bp")
                    nt = fw.tile([BLK, 2 * D], f32, tag="bn")
                    nc.sync.dma_start(out=ut[:], in_=rows_full.ap()[
                        ROLE_OFF[0] + BLK * j:ROLE_OFF[0] + BLK * (j + 1), :])
                    nc.sync.dma_start(out=pt_[:], in_=rows_full.ap()[
                        ROLE_OFF[1] + BLK * j:ROLE_OFF[1] + BLK * (j + 1), :])
                    nc.sync.dma_start(out=nt[:], in_=rows_full.ap()[
                        ROLE_OFF[2] + BLK * j:ROLE_OFF[2] + BLK * (j + 1), :])
                    nc.vector.tensor_mul(out=pt_[:], in0=ut[:], in1=pt_[:])
                    nc.vector.tensor_mul(out=nt[:], in0=ut[:], in1=nt[:])
                    nc.vector.reduce_sum(
                        out=P_t[:, j, :],
                        in_=pt_[:].rearrange("p (a f) -> p a f", a=2), axis=X)
                    nc.vector.reduce_sum(
                        out=N_t[:, j, :],
                        in_=nt[:].rearrange("p (a f) -> p a f", a=2), axis=X)
                x3i = fw.tile([BLK, BJ], f32, tag="x3i")
                x3p = fw.tile([BLK, BJ], f32, tag="x3p")
                x3t = fw.tile([BLK, BJ], f32, tag="x3t")
                nc.vector.tensor_sub(out=x3i[:], in0=P_t[:, :, 0],
                                     in1=N_t[:, :, 0])
                nc.vector.tensor_sub(out=x3p[:], in0=P_t[:, :, 1],
                                     in1=N_t[:, :, 1])
                nc.vector.tensor_add(out=x3t[:], in0=x3i[:], in1=x3p[:])
                mf = fr.tile([BLK, BJ], f32)
                nc.sync.dma_start(out=mf[:], in_=maskf_in[:, :])
                one_m = fw.tile([BLK, BJ], f32, tag="onem")
                nc.vector.tensor_scalar(out=one_m[:], in0=mf[:], scalar1=-1.0,
                                        scalar2=1.0, op0=OP.mult, op1=OP.add)
                S = 1.0 / 9.0

                def softplus(out, in_ap, scale, tag):
                    # out = ln(1 + exp(scale*in)) using Exp + Sqrt chain +
                    # one Newton step (no Ln/Softplus in the ACT tables).
                    w = fw.tile([BLK, BJ], f32, tag=tag + "w")
                    u = fw.tile([BLK, BJ], f32, tag=tag + "u")
                    sq = fw.tile([BLK, BJ], f32, tag=tag + "q")
                    nc.scalar.activation(out=w[:], in_=in_ap, func=FN.Exp,
                                         scale=scale)
                    nc.vector.tensor_scalar_add(out=w[:], in0=w[:], scalar1=1.0)
                    nc.scalar.sqrt(out=u[:], in_=w[:])
                    for _ in range(5):
                        nc.scalar.sqrt(out=u[:], in_=u[:])
                    # a = u - 1 ;  z0 = 64a - 32a^2
                    nc.vector.tensor_scalar_add(out=u[:], in0=u[:], scalar1=-1.0)
                    nc.vector.tensor_mul(out=sq[:], in0=u[:], in1=u[:])
                    nc.vector.tensor_scalar_mul(out=u[:], in0=u[:], scalar1=64.0)
                    nc.vector.tensor_scalar_mul(out=sq[:], in0=sq[:],
                                                scalar1=-32.0)
                    nc.vector.tensor_add(out=u[:], in0=u[:], in1=sq[:])
                    # newton: z1 = z0 + w*exp(-z0) - 1
                    nc.scalar.activation(out=sq[:], in_=u[:], func=FN.Exp,
                                         scale=-1.0)
                    nc.vector.tensor_mul(out=sq[:], in0=sq[:], in1=w[:])
                    nc.vector.tensor_scalar_add(out=sq[:], in0=sq[:],
                                                scalar1=-1.0)
                    nc.vector.tensor_add(out=out, in0=u[:], in1=sq[:])

                t1 = fw.tile([BLK, BJ], f32, tag="t1")
                t2 = fw.tile([BLK, BJ], f32, tag="t2")
                t3 = fw.tile([BLK, BJ], f32, tag="t3")
                t4 = fw.tile([BLK, BJ], f32, tag="t4")
                softplus(t1[:], x3i[:], -S, "a")
                softplus(t2[:], x3p[:], S, "b")
                softplus(t3[:], x3p[:], -S, "c")
                softplus(t4[:], x3t[:], -S, "d")
                nc.vector.tensor_mul(out=t1[:], in0=t1[:], in1=mf[:])
                nc.vector.tensor_mul(out=t2[:], in0=t2[:], in1=mf[:])
                nc.vector.tensor_mul(out=t3[:], in0=t3[:], in1=one_m[:])
                packed = fr.tile([BLK, 4], f32)
                nc.vector.reduce_sum(out=packed[:, 0:1], in_=t4[:], axis=X)
                nc.vector.reduce_sum(out=packed[:, 1:2], in_=t1[:], axis=X)
                nc.vector.tensor_add(out=t2[:], in0=t2[:], in1=t3[:])
                nc.vector.reduce_sum(out=packed[:, 2:3], in_=t2[:], axis=X)
                nc.vector.memset(packed[:, 3:4], 0.0)
                fin_ps = fps.tile([4, 1], f32, tag="fps")
                nc.tensor.matmul(out=fin_ps[:], lhsT=packed[:], rhs=ones_col[:],
                                 start=True, stop=True)
                cvec = fr.tile([4, 1], f32)
                nc.sync.dma_start(out=cvec[:], in_=cvec_in[:, :])
                fin = fr.tile([4, 1], f32)
                nc.scalar.mul(out=fin[:], in_=fin_ps[:], mul=cvec[:])
                # discrepancy from the AllReduced stats row
                srow = fw.tile([1, 2], f32, tag="sr")
                nc.sync.dma_start(
                    out=srow[:],
                    in_=rows_full.ap()[ROW_STATS:ROW_STATS + 1, 0:2])
                dcoef = fw.tile([1, 2], f32, tag="dc")
                nc.sync.dma_start(out=dcoef[:], in_=dcoef_in[:, :])
                nc.vector.tensor_mul(out=srow[:], in0=srow[:], in1=dcoef[:])
                dsum = fw.tile([1, 1], f32, tag="ds")
                nc.vector.reduce_sum(out=dsum[:], in_=srow[:], axis=X)
                # move dsum to partition 3 via a dram bounce, add into fin
                nc.sync.dma_start(out=scr_dram.ap()[:, :], in_=dsum[:])
                d3 = fw.tile([4, 1], f32, tag="d3")
                nc.vector.memset(d3[:], 0.0)
                nc.sync.dma_start(out=d3[3:4, :], in_=scr_dram.ap()[:, :])
                nc.vector.tensor_add(out=fin[:], in0=fin[:], in1=d3[:])
                nc.sync.dma_start(out=out4.ap(), in_=fin[:, 0])

    return nc


# ==================================================================== kernel()
_CACHE = {}


def _build_program(n_item_u, n_user_u):
    import concourse.bacc as bacc
    key = (n_item_u, n_user_u)
    if key not in _CACHE:
        nc = bacc.Bacc("TRN2", target_bir_lowering=False, debug=False,
                       num_devices=CORES, num_swdge_queues=NQ)
        _build(nc, n_item_u, n_user_u)
        nc.compile()
        _CACHE[key] = nc
    return _CACHE[key]


def kernel(**inputs):
    from concourse.bass_utils import run_bass_kernel_spmd

    emb_int = np.asarray(inputs["emb_int"], np.float32)
    emb_pop = np.asarray(inputs["emb_pop"], np.float32)
    user = np.asarray(inputs["user"]).astype(np.int64)
    item_p = np.asarray(inputs["item_p"]).astype(np.int64)
    item_n = np.asarray(inputs["item_n"]).astype(np.int64)
    mask = np.asarray(inputs["mask"])
    src = np.asarray(inputs["src"]).astype(np.int64)
    dst = np.asarray(inputs["dst"]).astype(np.int64)

    t0_full = np.zeros((NPAD, 2 * D), np.float32)
    t0_full[:N, :D] = emb_int
    t0_full[:N, D:] = emb_pop

    cores_meta = [_prep_core(c, src, dst, t0_full) for c in range(CORES)]
    bidx16, ownmask, ind, n_item_u, n_user_u = _prep_batch(user, item_p,
                                                           item_n)
    import ml_dtypes
    iota8 = np.tile(np.arange(BLK, dtype=np.float32),
                    (BLK, 8)).reshape(BLK, 8 * BLK).astype(ml_dtypes.bfloat16)
    maskf = np.ascontiguousarray(
        mask.reshape(-1).astype(np.float32).reshape(BJ, BLK).T)

    nc = _build_program(n_item_u, n_user_u)

    in_maps = []
    for c in range(CORES):
        m = cores_meta[c]
        in_maps.append({
            "t0": t0_full,
            "t0s": m["t0_slice"],
            "idx16": m["idx16"],
            "dstloc": m["dstloc"].astype(ml_dtypes.bfloat16),
            "deg": m["deg"],
            "ind": ind[c],
            "iota8": iota8,
            "bidx16": bidx16[c],
            "ownmask": ownmask[c],
            "maskf": maskf,
            "cvec": np.array([[1.0 / B], [INT_W / B], [POP_W / B], [0.0]],
                             np.float32),
            "dcoef": np.array([[-DIS_PEN / (n_item_u * D * 9.0),
                                -DIS_PEN / (n_user_u * D * 9.0)]], np.float32),
        })

    trace = bool(int(os.environ.get("LGN_TRACE", "0")))
    res = run_bass_kernel_spmd(nc, in_maps, core_ids=list(range(CORES)),
                               trace=trace)
    if trace and res.exec_time_ns is not None:
        print(f"HW exec time: {res.exec_time_ns} ns")
    return res.results[0]["out4"]
